# revision 1
# baseline (speedup 1.0000x reference)
"""Trainium2 Bass kernel for nn_CoeusBlockOptimized — 8-core SPMD.

Sharding: the parallel phase is feature/head-sharded (core c owns attention
heads (2c, 2c+1) and recurrence D-shard [128c, 128c+128) for ALL tokens); the
tail is token-sharded (core c owns flattened token block [512c, 512c+512)).
Cross-core traffic: one bf16 AllGather of h^T (1 MB/rank) and one bf16
AllToAll carrying attention output + recurrence state (2 MB/rank).

All activations are kept transposed (feature rows on partitions, tokens on
the free axis); per-token reductions (rms/layernorm/softmax denominators) use
ones-vector matmuls on the PE plus a K=1 broadcast matmul back to 128
partitions.  Matmuls run in bf16 (fp32 PSUM accumulation); the sequential
recurrence uses the VectorE tensor_tensor_scan instruction in fp32.
"""
import sys
import os

for _p in ("/opt/trn_rl_repo", "/root/.axon_site/_ro/trn_rl_repo"):
    if os.path.isdir(_p) and _p not in sys.path:
        sys.path.insert(0, _p)

import numpy as np
import ml_dtypes

import concourse.bass as bass
import concourse.tile as tile
from concourse import mybir, bacc
from concourse.bass_utils import run_bass_kernel_spmd
from concourse.masks import make_identity

BF = ml_dtypes.bfloat16
F32 = mybir.dt.float32
BF16 = mybir.dt.bfloat16
AF = mybir.ActivationFunctionType
OP = mybir.AluOpType

NC = 8
B, T, D = 2, 2048, 1024
H, HD, HID, M = 16, 64, 3072, 256
FREQ = 8
TB = 512            # tokens per core block
KT = D // 128       # 8 d-tiles
NH = HID // 128     # 24
NT = B * T          # 4096 tokens

_PROG_CACHE = {}


def _tw(w):
    """(Mout, Kin) weight -> (128, Kin/128, Mout) bf16 lhsT-tile layout."""
    k, m = w.shape[1], w.shape[0]
    assert k % 128 == 0
    return np.ascontiguousarray(
        w.T.reshape(k // 128, 128, m).transpose(1, 0, 2)).astype(BF)


def _cw(v):
    """(1024,) vector -> (128, 8) fp32 per-partition column layout."""
    return np.ascontiguousarray(v.reshape(-1, 128).T).astype(np.float32)


def _head_perm(h):
    base = h * HD
    return [base + i for i in range(0, HD, 2)] + [base + i for i in range(1, HD, 2)]


def _build_program():
    nc = bacc.Bacc("TRN2", target_bir_lowering=False, debug=False, num_devices=NC)

    def din(name, shape, dt):
        return nc.dram_tensor(name, list(shape), dt, kind="ExternalInput")

    # per-core inputs
    xt = din("xt", (128, KT, TB), F32)
    wq_t = din("wq_t", (128, KT, 128), BF16)
    wk_t = din("wk_t", (128, KT, 128), BF16)
    wv_t = din("wv_t", (128, KT, 128), BF16)
    wg_t = din("wg_t", (128, KT, 128), BF16)
    wu_t = din("wu_t", (128, KT, 128), BF16)
    gate_b = din("gate_b", (128, 1), F32)
    bmask = din("bmask", (128, 2 * M), F32)
    # shared inputs
    cos4 = din("cos4", (128, T), F32)
    sin4 = din("sin4", (128, T), F32)
    maskT_in = din("maskT", (128, 4, TB), BF16)
    norm_cols = din("norm_cols", (128, 8, KT), F32)   # packed norm/bias columns
    mgb = din("mgb", (1, 1), F32)
    wo_t = din("wo_t", (128, KT, D), BF16)
    wog_t = din("wog_t", (128, KT, D), BF16)
    mcomp_t = din("mcomp_t", (128, KT, M), BF16)
    mq_t = din("mq_t", (128, KT, M), BF16)
    mk_t = din("mk_t", (128, 2, M), BF16)
    mv_t = din("mv_t", (128, 2, D), BF16)
    mg_t = din("mg_t", (128, KT, 1), BF16)
    lgr_t = din("lgr_t", (128, KT, D), BF16)
    lga_t = din("lga_t", (128, KT, D), BF16)
    w1a_t = din("w1a_t", (128, KT, D), BF16)
    w1b_t = din("w1b_t", (128, KT, D), BF16)
    w2_t = din("w2_t", (128, KT, D), BF16)
    rw1_t = din("rw1_t", (128, KT, HID), BF16)
    rw3_t = din("rw3_t", (128, KT, HID), BF16)
    rw2_t = din("rw2_t", (128, NH, D), BF16)

    out_ext = nc.dram_tensor("out", [128, KT, TB], F32, kind="ExternalOutput")

    # norm_cols packing order
    NCOL = {"anorm": 0, "cnorm": 1, "ffn": 2, "rn": 3, "b1": 4, "lnw": 5,
            "lnb": 6, "b2": 7}

    with tile.TileContext(nc) as tc:
        with tc.tile_pool(name="dram", bufs=1, space="DRAM") as dram, \
             tc.tile_pool(name="const", bufs=1) as const, \
             tc.tile_pool(name="resid", bufs=1) as resid, \
             tc.tile_pool(name="tmp", bufs=3) as tmp, \
             tc.tile_pool(name="rows", bufs=3) as rows, \
             tc.tile_pool(name="ps_mm", bufs=3, space="PSUM") as ps_mm, \
             tc.tile_pool(name="ps_bc", bufs=2, space="PSUM") as ps_bc, \
             tc.tile_pool(name="ps_row", bufs=2, space="PSUM") as ps_row:

            # ---- DRAM comm buffers ----
            ag_in = dram.tile([KT, 128, TB], BF16)
            ag_out = dram.tile([NC, KT, 128, TB], BF16, addr_space="Shared")
            a2a_in = dram.tile([NC, 2, 128, TB], BF16)
            a2a_out = dram.tile([NC, 2, 128, TB], BF16)

            # ---- constants ----
            ones_col = const.tile([128, 1], BF16)
            ones_row = const.tile([1, 128], BF16)
            ident = const.tile([128, 128], BF16)
            nc.any.memset(ones_col[:], 1.0)
            nc.any.memset(ones_row[:], 1.0)
            make_identity(nc, ident[:])
            cos_sb = const.tile([128, T], F32)
            sin_sb = const.tile([128, T], F32)
            ncols = const.tile([128, 8, KT], F32)
            gate_b_sb = const.tile([128, 1], F32)
            mgb_sb = const.tile([1, 1], F32)
            eps6_sb = const.tile([1, 1], F32)
            eps5_sb = const.tile([1, 1], F32)
            nc.any.memset(eps6_sb[:], 1e-6)
            nc.any.memset(eps5_sb[:], 1e-5)
            nc.sync.dma_start(cos_sb[:], cos4[:])
            nc.sync.dma_start(sin_sb[:], sin4[:])
            nc.sync.dma_start(ncols[:], norm_cols[:])
            nc.sync.dma_start(gate_b_sb[:], gate_b[:])
            nc.sync.dma_start(mgb_sb[:], mgb[:])

            def ncol(nm, k):
                return ncols[:, NCOL[nm], k:k + 1]

            wq_sb = const.tile([128, KT, 128], BF16)
            wk_sb = const.tile([128, KT, 128], BF16)
            wv_sb = const.tile([128, KT, 128], BF16)
            wg_sb = const.tile([128, KT, 128], BF16)
            wu_sb = const.tile([128, KT, 128], BF16)
            mcomp_sb = const.tile([128, KT, M], BF16)
            mq_sb = const.tile([128, KT, M], BF16)
            mk_sb = const.tile([128, 2, M], BF16)
            mv_sb = const.tile([128, 2, D], BF16)
            mg_sb = const.tile([128, KT, 1], BF16)
            for sb, t_in in ((wq_sb, wq_t), (wk_sb, wk_t), (wv_sb, wv_t),
                             (wg_sb, wg_t), (wu_sb, wu_t), (mcomp_sb, mcomp_t),
                             (mq_sb, mq_t), (mk_sb, mk_t), (mv_sb, mv_t),
                             (mg_sb, mg_t)):
                nc.sync.dma_start(sb[:], t_in[:])

            # ---- long-lived activations (whole-kernel) ----
            epis = resid.tile([128, KT, TB], BF16)
            x2 = resid.tile([128, KT, TB], F32)

            # ---------- shared helpers ----------
            def rms_rsqrt_row(src_getter, eps_ap, nk=KT, width=TB):
                ss = ps_row.tile([1, width], F32, tag="psrow", name="ss")
                for k in range(nk):
                    sq = tmp.tile([128, width], BF16, tag="tb16", name="sq")
                    nc.vector.tensor_tensor(sq[:], src_getter(k), src_getter(k),
                                            OP.mult)
                    nc.tensor.matmul(ss[:], ones_col[:], sq[:],
                                     start=(k == 0), stop=(k == nk - 1))
                ms = rows.tile([1, width], F32, tag="row32", name="ms")
                nc.scalar.activation(ms[:], ss[:], AF.Identity,
                                     bias=eps_ap[0:1, 0:1], scale=1.0 / (nk * 128))
                rc = rows.tile([1, width], F32, tag="row32", name="rc")
                nc.vector.reciprocal(rc[:], ms[:])
                rs_row = rows.tile([1, width], BF16, tag="row16", name="rsr")
                nc.scalar.activation(rs_row[:], rc[:], AF.Sqrt)
                return rs_row

            def bcast_row(row_bf, width=TB, np_=128):
                bc = ps_bc.tile([np_, width], F32, tag="bc", name="bc")
                nc.tensor.matmul(bc[:], ones_row[0:1, 0:np_], row_bf[:])
                return bc

            # ================= phase 0: local h^T + AllGather =================
            with tc.tile_pool(name="htloc", bufs=1) as htloc_pool:
                ht_loc = htloc_pool.tile([128, KT, TB], BF16)
                with tc.tile_pool(name="ph0", bufs=1) as ph0:
                    xt_sb = ph0.tile([128, KT, TB], F32)
                    nc.sync.dma_start(xt_sb[:], xt[:])
                    rsq = rms_rsqrt_row(lambda k: xt_sb[:, k, :], eps6_sb)
                    bc = bcast_row(rsq)
                    for k in range(KT):
                        nc.vector.scalar_tensor_tensor(
                            ht_loc[:, k, :], xt_sb[:, k, :], ncol("anorm", k),
                            bc[:], OP.mult, OP.mult)
                        nc.sync.dma_start(ag_in[k], ht_loc[:, k, :])
                if True:
                    nc.gpsimd.collective_compute(
                        "AllGather", OP.bypass,
                        replica_groups=[list(range(NC))],
                        ins=[ag_in.opt()], outs=[ag_out.opt()])

                    with tc.tile_pool(name="ht", bufs=1) as ht_pool:
                        ht = ht_pool.tile([128, KT, NC, TB], BF16)
                        for k in range(KT):
                            nc.sync.dma_start(
                                ht[:, k, :, :],
                                ag_out[:, k, :, :].rearrange("blk p t -> p blk t"))

                        # ============ phase 1: attention ============
                        with tc.tile_pool(name="attn", bufs=1) as attn:
                            mask_sb = attn.tile([128, 4, TB], BF16)
                            nc.sync.dma_start(mask_sb[:], maskT_in[:])
                            for b in range(B):
                                qT = attn.tile([128, T], BF16, tag="qT",
                                               name=f"qT{b}")
                                kTt = attn.tile([128, T], BF16, tag="kT",
                                                name=f"kT{b}")
                                vn = attn.tile([128, 16, 130], BF16, tag="vn",
                                               name=f"vn{b}")
                                nc.any.memset(vn[:, :, 64:65], 1.0)
                                nc.any.memset(vn[:, :, 129:130], 1.0)

                                for dst, wsb in ((qT, wq_sb), (kTt, wk_sb)):
                                    for nb in range(4):
                                        cols = slice(nb * TB, (nb + 1) * TB)
                                        ps = ps_mm.tile([128, TB], F32, tag="mm",
                                                        name="qk_ps")
                                        for k in range(KT):
                                            nc.tensor.matmul(
                                                ps[:], wsb[:, k, :],
                                                ht[:, k, 4 * b + nb, :],
                                                start=(k == 0),
                                                stop=(k == KT - 1))
                                        m1 = tmp.tile([128, TB], F32, tag="tf32",
                                                      name="rot1")
                                        m2 = ps_bc.tile([128, TB], F32, tag="bc",
                                                        name="rot2")
                                        nc.vector.tensor_tensor(
                                            m1[:], ps[:], cos_sb[:, cols], OP.mult)
                                        nc.vector.tensor_tensor(
                                            m2[:], ps[:], sin_sb[:, cols], OP.mult)
                                        for h2 in range(2):
                                            be = 64 * h2
                                            nc.vector.tensor_tensor(
                                                dst[be:be + 32, cols],
                                                m1[be:be + 32, :],
                                                m2[be + 32:be + 64, :],
                                                OP.subtract)
                                            nc.vector.tensor_tensor(
                                                dst[be + 32:be + 64, cols],
                                                m1[be + 32:be + 64, :],
                                                m2[be:be + 32, :], OP.add)

                                for tt in range(16):
                                    nb, sub = tt // 4, tt % 4
                                    ps = ps_mm.tile([128, 128], F32, tag="mm",
                                                    name="v_ps")
                                    for k in range(KT):
                                        nc.tensor.matmul(
                                            ps[:],
                                            ht[:, k, 4 * b + nb,
                                               sub * 128:(sub + 1) * 128],
                                            wv_sb[:, k, :],
                                            start=(k == 0), stop=(k == KT - 1))
                                    nc.scalar.copy(vn[:, tt, 0:64], ps[:, 0:64])
                                    nc.scalar.copy(vn[:, tt, 65:129],
                                                   ps[:, 64:128])

                                for h2 in range(2):
                                    be = 64 * h2
                                    for qb in range(4):
                                        qcols = slice(qb * TB, (qb + 1) * TB)
                                        nlive = 4 * qb + 4
                                        pt = attn.tile([128, 16, TB], BF16,
                                                       tag="pt", name="pt",
                                                       bufs=2)
                                        for ki in range(nlive):
                                            sps = ps_mm.tile([128, TB], F32,
                                                             tag="mm", name="s_ps")
                                            nc.tensor.matmul(
                                                sps[:],
                                                kTt[be:be + 64,
                                                    ki * 128:(ki + 1) * 128],
                                                qT[be:be + 64, qcols],
                                                start=True, stop=True)
                                            nc.scalar.activation(
                                                pt[:, ki, :], sps[:], AF.Exp,
                                                scale=0.125)
                                            if ki >= 4 * qb:
                                                nc.vector.tensor_tensor(
                                                    pt[:, ki, :], pt[:, ki, :],
                                                    mask_sb[:, ki - 4 * qb, :],
                                                    OP.mult)
                                        av = ps_mm.tile([65, TB], F32, tag="mm",
                                                        name="av_ps")
                                        for ki in range(nlive):
                                            nc.tensor.matmul(
                                                av[:],
                                                vn[:, ki, 65 * h2:65 * h2 + 65],
                                                pt[:, ki, :], start=(ki == 0),
                                                stop=(ki == nlive - 1))
                                        rd = rows.tile([1, TB], F32, tag="row32",
                                                       name="rd")
                                        nc.vector.reciprocal(rd[:], av[64:65, :])
                                        rdb = rows.tile([1, TB], BF16,
                                                        tag="row16", name="rdb")
                                        nc.scalar.copy(rdb[:], rd[:])
                                        bcd = bcast_row(rdb, np_=64)
                                        bsb = tmp.tile([64, TB], BF16, tag="t64",
                                                       name="bsb", bufs=4)
                                        nc.scalar.copy(bsb[:], bcd[:])
                                        an_t = tmp.tile([64, TB], BF16,
                                                        tag="t64", name="an_t",
                                                        bufs=4)
                                        nc.vector.tensor_tensor(
                                            an_t[:], av[0:64, :], bsb[:], OP.mult)
                                        nc.sync.dma_start(
                                            a2a_in[4 * b + qb, 0,
                                                   be:be + 64, :], an_t[:])

                        # ============ phase 2: recurrence ============
                        with tc.tile_pool(name="scan", bufs=1) as scan_pool:
                            gate_sb = scan_pool.tile([128, NC, TB], F32)
                            u_sb = scan_pool.tile([128, NC, TB], F32)
                            hst_sb = scan_pool.tile([128, NC * TB], BF16)
                            for blk in range(NC):
                                psg = ps_mm.tile([128, TB], F32, tag="mm",
                                                 name="g_ps")
                                for k in range(KT):
                                    nc.tensor.matmul(psg[:], wg_sb[:, k, :],
                                                     ht[:, k, blk, :],
                                                     start=(k == 0),
                                                     stop=(k == KT - 1))
                                nc.scalar.activation(gate_sb[:, blk, :], psg[:],
                                                     AF.Sigmoid,
                                                     bias=gate_b_sb[:])
                                psu = ps_mm.tile([128, TB], F32, tag="mm",
                                                 name="u_ps")
                                for k in range(KT):
                                    nc.tensor.matmul(psu[:], wu_sb[:, k, :],
                                                     ht[:, k, blk, :],
                                                     start=(k == 0),
                                                     stop=(k == KT - 1))
                                nc.scalar.activation(u_sb[:, blk, :], psu[:],
                                                     AF.Silu)
                            g2d = gate_sb[:].rearrange("p a b -> p (a b)")
                            u2d = u_sb[:].rearrange("p a b -> p (a b)")
                            for b in range(B):
                                cols = slice(b * T, (b + 1) * T)
                                nc.vector.tensor_tensor_scan(
                                    hst_sb[:, cols], g2d[:, cols], u2d[:, cols],
                                    0.0, OP.mult, OP.add)
                            for j in range(NC):
                                nc.sync.dma_start(
                                    a2a_in[j, 1], hst_sb[:, j * TB:(j + 1) * TB])

                        nc.gpsimd.collective_compute(
                            "AllToAll", OP.bypass,
                            replica_groups=[list(range(NC))],
                            ins=[a2a_in.opt()], outs=[a2a_out.opt()])

                        # ============ phase 3: episodic (overlaps A2A) =========
                        with tc.tile_pool(name="ep", bufs=1) as ep:
                            bmask_sb = ep.tile([128, 2 * M], F32)
                            nc.sync.dma_start(bmask_sb[:], bmask[:])
                            memk = ep.tile([128, 2, 2 * M], BF16)
                            ktm = ep.tile([128, 2, 2 * M], BF16)
                            qtm = ep.tile([128, 2, TB], BF16)
                            a_sb = ep.tile([128, 4, 2 * M], BF16)
                            at_sb = ep.tile([128, 4, TB], BF16)
                            sn_sb = ep.tile([128, 4, D], BF16)
                            mo_sb = ep.tile([128, KT, TB], BF16)
                            moc = ep.tile([128, 2, TB], BF16)
                            gbc = ep.tile([128, TB], BF16)

                            for mi in range(2):
                                ps = ps_mm.tile([128, 2 * M], F32, tag="mm",
                                                name="mk_ps")
                                for k in range(KT):
                                    nc.tensor.matmul(
                                        ps[:],
                                        mcomp_sb[:, k, mi * 128:(mi + 1) * 128],
                                        ht[:, k, :, 0:TB:FREQ],
                                        start=(k == 0), stop=(k == KT - 1))
                                nc.scalar.copy(memk[:, mi, :], ps[:])
                            for mo in range(2):
                                ps = ps_mm.tile([128, 2 * M], F32, tag="mm",
                                                name="kt_ps")
                                for mi in range(2):
                                    nc.tensor.matmul(
                                        ps[:],
                                        mk_sb[:, mi, mo * 128:(mo + 1) * 128],
                                        memk[:, mi, :], start=(mi == 0),
                                        stop=(mi == 1))
                                nc.scalar.copy(ktm[:, mo, :], ps[:])
                            for mi in range(2):
                                ps = ps_mm.tile([128, TB], F32, tag="mm",
                                                name="q_ps")
                                for k in range(KT):
                                    nc.tensor.matmul(
                                        ps[:],
                                        mq_sb[:, k, mi * 128:(mi + 1) * 128],
                                        ht_loc[:, k, :], start=(k == 0),
                                        stop=(k == KT - 1))
                                nc.scalar.copy(qtm[:, mi, :], ps[:])
                            for tt in range(4):
                                ps = ps_mm.tile([128, 2 * M], F32, tag="mm",
                                                name="sc_ps")
                                for mi in range(2):
                                    nc.tensor.matmul(
                                        ps[:],
                                        qtm[:, mi, tt * 128:(tt + 1) * 128],
                                        ktm[:, mi, :], start=(mi == 0),
                                        stop=(mi == 1))
                                sm = tmp.tile([128, 2 * M], F32, tag="tf32",
                                              name="sm")
                                nc.vector.tensor_tensor(sm[:], ps[:], bmask_sb[:],
                                                        OP.add)
                                den = rows.tile([128, 1], F32, tag="den",
                                                name="den")
                                nc.scalar.activation(a_sb[:, tt, :], sm[:],
                                                     AF.Exp, scale=1.0 / 16.0,
                                                     accum_out=den[:])
                                rden = rows.tile([128, 1], F32, tag="den",
                                                 name="rden")
                                nc.vector.reciprocal(rden[:], den[:])
                                nc.vector.tensor_scalar_mul(
                                    a_sb[:, tt, :], a_sb[:, tt, :], rden[:])
                            for st in range(4):
                                for k in range(KT):
                                    pst = ps_mm.tile([128, 128], BF16, tag="mm",
                                                     name="tr_ps")
                                    nc.tensor.transpose(
                                        pst[:],
                                        ht[:, k, 2 * st:2 * st + 2, 0:TB:FREQ],
                                        ident[:])
                                    nc.scalar.copy(
                                        sn_sb[:, st, k * 128:(k + 1) * 128],
                                        pst[:])
                                for tt in range(4):
                                    pst = ps_mm.tile([128, 128], BF16, tag="mm",
                                                     name="tr2_ps")
                                    nc.tensor.transpose(
                                        pst[:],
                                        a_sb[:, tt, st * 128:(st + 1) * 128],
                                        ident[:])
                                    nc.scalar.copy(
                                        at_sb[:, st, tt * 128:(tt + 1) * 128],
                                        pst[:])
                            for dm in range(KT):
                                ps = ps_mm.tile([128, TB], F32, tag="mm",
                                                name="mo_ps")
                                for st in range(4):
                                    nc.tensor.matmul(
                                        ps[:],
                                        sn_sb[:, st, dm * 128:(dm + 1) * 128],
                                        at_sb[:, st, :], start=(st == 0),
                                        stop=(st == 3))
                                nc.scalar.copy(mo_sb[:, dm, :], ps[:])
                            for mi in range(2):
                                ps = ps_mm.tile([128, TB], F32, tag="mm",
                                                name="moc_ps")
                                for k in range(KT):
                                    nc.tensor.matmul(
                                        ps[:],
                                        mcomp_sb[:, k, mi * 128:(mi + 1) * 128],
                                        mo_sb[:, k, :], start=(k == 0),
                                        stop=(k == KT - 1))
                                nc.scalar.copy(moc[:, mi, :], ps[:])
                            psg2 = ps_row.tile([1, TB], F32, tag="psrow",
                                               name="g_psr")
                            for k in range(KT):
                                nc.tensor.matmul(psg2[:], mg_sb[:, k, :],
                                                 ht_loc[:, k, :],
                                                 start=(k == 0),
                                                 stop=(k == KT - 1))
                            grow = rows.tile([1, TB], BF16, tag="row16",
                                             name="grow")
                            nc.scalar.activation(grow[:], psg2[:], AF.Sigmoid,
                                                 bias=mgb_sb[0:1, 0:1])
                            gb = bcast_row(grow)
                            nc.scalar.copy(gbc[:], gb[:])
                            for dm in range(KT):
                                ps = ps_mm.tile([128, TB], F32, tag="mm",
                                                name="mv_ps")
                                for mi in range(2):
                                    nc.tensor.matmul(
                                        ps[:],
                                        mv_sb[:, mi, dm * 128:(dm + 1) * 128],
                                        moc[:, mi, :], start=(mi == 0),
                                        stop=(mi == 1))
                                nc.vector.tensor_tensor(epis[:, dm, :], ps[:],
                                                        gbc[:], OP.mult)
            # ht, ht_loc, ph0 pools closed here

            # ================= phase 4: tail (token-parallel) =================
            with tc.tile_pool(name="tlg", bufs=1) as tlg, \
                 tc.tile_pool(name="wt", bufs=2) as wt:
                atnT = tlg.tile([128, KT, TB], BF16)
                hstT = tlg.tile([128, KT, TB], BF16)
                nc.sync.dma_start(atnT[:],
                                  a2a_out[:, 0].rearrange("i p t -> p i t"))
                nc.sync.dma_start(hstT[:],
                                  a2a_out[:, 1].rearrange("i p t -> p i t"))

                def mm_chain(w_dram, rhs_fn, evict, nk=KT, nm=KT):
                    wsb = wt.tile([128, nk, D], BF16, tag="w", name="wstream")
                    nc.sync.dma_start(wsb[:], w_dram[:])
                    for m in range(nm):
                        ps = ps_mm.tile([128, TB], F32, tag="mm", name="c_ps")
                        for k in range(nk):
                            nc.tensor.matmul(ps[:],
                                             wsb[:, k, m * 128:(m + 1) * 128],
                                             rhs_fn(k), start=(k == 0),
                                             stop=(k == nk - 1))
                        evict(m, ps)

                ol = tlg.tile([128, KT, TB], BF16)
                mm_chain(wo_t, lambda k: atnT[:, k, :],
                         lambda m, ps: nc.scalar.copy(ol[:, m, :], ps[:]))

                nT = tlg.tile([128, KT, TB], BF16, tag="seq8", name="nT")
                rsq2 = rms_rsqrt_row(lambda k: hstT[:, k, :], eps6_sb)
                bc2 = bcast_row(rsq2)
                for k in range(KT):
                    nc.vector.scalar_tensor_tensor(
                        nT[:, k, :], hstT[:, k, :], ncol("cnorm", k), bc2[:],
                        OP.mult, OP.mult)
                og = tlg.tile([128, KT, TB], BF16)
                mm_chain(wog_t, lambda k: nT[:, k, :],
                         lambda m, ps: nc.scalar.copy(og[:, m, :], ps[:]))

                hrnn = tlg.tile([128, KT, TB], BF16, tag="hio", name="hrnn", bufs=2)
                mm_chain(lgr_t, lambda k: og[:, k, :],
                         lambda m, ps: nc.scalar.copy(hrnn[:, m, :], ps[:]))
                hatt = tlg.tile([128, KT, TB], BF16, tag="hio", name="hatt", bufs=2)
                mm_chain(lga_t, lambda k: ol[:, k, :],
                         lambda m, ps: nc.scalar.copy(hatt[:, m, :], ps[:]))

                t1f = tlg.tile([128, KT, TB], F32)
                t1b = tlg.tile([128, KT, TB], BF16, tag="seq8", name="t1b")
                w1a_sb = wt.tile([128, KT, D], BF16, tag="w", name="w1a_sb")
                w1b_sb = wt.tile([128, KT, D], BF16, tag="w", name="w1b_sb")
                nc.sync.dma_start(w1a_sb[:], w1a_t[:])
                nc.sync.dma_start(w1b_sb[:], w1b_t[:])
                for m in range(KT):
                    ps = ps_mm.tile([128, TB], F32, tag="mm", name="t1_ps")
                    for k in range(KT):
                        nc.tensor.matmul(ps[:],
                                         w1a_sb[:, k, m * 128:(m + 1) * 128],
                                         hrnn[:, k, :], start=(k == 0),
                                         stop=False)
                    for k in range(KT):
                        nc.tensor.matmul(ps[:],
                                         w1b_sb[:, k, m * 128:(m + 1) * 128],
                                         hatt[:, k, :], start=False,
                                         stop=(k == KT - 1))
                    nc.scalar.activation(t1f[:, m, :], ps[:], AF.Identity,
                                         bias=ncol("b1", m))
                    nc.vector.tensor_copy(t1b[:, m, :], t1f[:, m, :])

                # layernorm stats
                ssum = ps_row.tile([1, TB], F32, tag="psrow", name="ssum")
                for k in range(KT):
                    nc.tensor.matmul(ssum[:], ones_col[:], t1b[:, k, :],
                                     start=(k == 0), stop=(k == KT - 1))
                ssq = ps_row.tile([1, TB], F32, tag="psrow", name="ssq")
                for k in range(KT):
                    sq = tmp.tile([128, TB], BF16, tag="tb16", name="sq2")
                    nc.vector.tensor_tensor(sq[:], t1b[:, k, :], t1b[:, k, :],
                                            OP.mult)
                    nc.tensor.matmul(ssq[:], ones_col[:], sq[:],
                                     start=(k == 0), stop=(k == KT - 1))
                mu = rows.tile([1, TB], F32, tag="row32", name="mu")
                nc.scalar.activation(mu[:], ssum[:], AF.Identity, scale=1.0 / D)
                mub = rows.tile([1, TB], BF16, tag="row16", name="mub")
                nc.scalar.copy(mub[:], mu[:])
                mu2 = rows.tile([1, TB], F32, tag="row32", name="mu2")
                nc.vector.tensor_tensor(mu2[:], mu[:], mu[:], OP.mult)
                ex2 = rows.tile([1, TB], F32, tag="row32", name="ex2")
                nc.scalar.activation(ex2[:], ssq[:], AF.Identity, scale=1.0 / D)
                varr = rows.tile([1, TB], F32, tag="row32", name="varr")
                nc.vector.tensor_tensor(varr[:], ex2[:], mu2[:], OP.subtract)
                vre = rows.tile([1, TB], F32, tag="row32", name="vre")
                nc.scalar.activation(vre[:], varr[:], AF.Identity,
                                     bias=eps5_sb[0:1, 0:1])
                rcv = rows.tile([1, TB], F32, tag="row32", name="rcv")
                nc.vector.reciprocal(rcv[:], vre[:])
                rsl = rows.tile([1, TB], BF16, tag="row16", name="rsl")
                nc.scalar.activation(rsl[:], rcv[:], AF.Sqrt)
                bc_mu = bcast_row(mub)
                mu_sb = tmp.tile([128, TB], F32, tag="tf32", name="mu_sb")
                nc.scalar.copy(mu_sb[:], bc_mu[:])
                bc_rs = bcast_row(rsl)
                zt = tlg.tile([128, KT, TB], BF16, tag="seq8", name="zt")
                for k in range(KT):
                    d1 = tmp.tile([128, TB], F32, tag="tf32", name="d1")
                    nc.vector.tensor_tensor(d1[:], t1f[:, k, :], mu_sb[:],
                                            OP.subtract)
                    d2 = tmp.tile([128, TB], F32, tag="tf32", name="d2")
                    nc.vector.scalar_tensor_tensor(d2[:], d1[:], ncol("lnw", k),
                                                   bc_rs[:], OP.mult, OP.mult)
                    d3 = tmp.tile([128, TB], F32, tag="tf32", name="d3")
                    nc.vector.tensor_scalar_add(d3[:], d2[:], ncol("lnb", k))
                    nc.scalar.activation(zt[:, k, :], d3[:], AF.Silu)
                g2t = tlg.tile([128, KT, TB], BF16)
                mm_chain(w2_t, lambda k: zt[:, k, :],
                         lambda m, ps: nc.scalar.activation(
                             g2t[:, m, :], ps[:], AF.Sigmoid, bias=ncol("b2", m)))

                for k in range(KT):
                    xtk = tmp.tile([128, TB], F32, tag="tf32", name="xtk")
                    nc.sync.dma_start(xtk[:], xt[:, k, :])
                    mx1 = tmp.tile([128, TB], F32, tag="tf32", name="mx1")
                    nc.vector.tensor_tensor(mx1[:], ol[:, k, :], og[:, k, :],
                                            OP.subtract)
                    mx2 = tmp.tile([128, TB], F32, tag="tf32", name="mx2")
                    nc.vector.tensor_tensor(mx2[:], g2t[:, k, :], mx1[:],
                                            OP.mult)
                    mx3 = tmp.tile([128, TB], F32, tag="tf32", name="mx3")
                    nc.vector.tensor_tensor(mx3[:], og[:, k, :], mx2[:], OP.add)
                    mx4 = tmp.tile([128, TB], F32, tag="tf32", name="mx4")
                    nc.vector.tensor_tensor(mx4[:], xtk[:], mx3[:], OP.add)
                    nc.vector.tensor_tensor(x2[:, k, :], mx4[:], epis[:, k, :],
                                            OP.add)

            # ================= phase 5: reasoning (SwiGLU x2) =================
            with tc.tile_pool(name="trs", bufs=1) as trs, \
                 tc.tile_pool(name="wr", bufs=1) as wr:
                rs = trs.tile([128, KT, TB], F32)
                rsq3 = rms_rsqrt_row(lambda k: x2[:, k, :], eps6_sb)
                bc3 = bcast_row(rsq3)
                for k in range(KT):
                    nc.vector.scalar_tensor_tensor(
                        rs[:, k, :], x2[:, k, :], ncol("ffn", k), bc3[:],
                        OP.mult, OP.mult)
                for it in range(2):
                    nrm = trs.tile([128, KT, TB], BF16, tag="nrm",
                                   name=f"nrm{it}")
                    rsq4 = rms_rsqrt_row(lambda k: rs[:, k, :], eps6_sb)
                    bc4 = bcast_row(rsq4)
                    for k in range(KT):
                        nc.vector.scalar_tensor_tensor(
                            nrm[:, k, :], rs[:, k, :], ncol("rn", k), bc4[:],
                            OP.mult, OP.mult)
                    w1sb = wr.tile([128, KT, HID], BF16, tag="rws",
                                   name=f"w1sb{it}")
                    nc.sync.dma_start(w1sb[:], rw1_t[:])
                    asb = trs.tile([128, NH, TB], BF16, tag="asb",
                                   name=f"asb{it}")
                    for m in range(NH):
                        ps = ps_mm.tile([128, TB], F32, tag="mm", name="a_ps")
                        for k in range(KT):
                            nc.tensor.matmul(
                                ps[:], w1sb[:, k, m * 128:(m + 1) * 128],
                                nrm[:, k, :], start=(k == 0), stop=(k == KT - 1))
                        nc.scalar.activation(asb[:, m, :], ps[:], AF.Silu)
                    w3sb = wr.tile([128, KT, HID], BF16, tag="rws",
                                   name=f"w3sb{it}")
                    nc.sync.dma_start(w3sb[:], rw3_t[:])
                    absb = trs.tile([128, NH, TB], BF16, tag="absb",
                                    name=f"absb{it}")
                    for m in range(NH):
                        ps = ps_mm.tile([128, TB], F32, tag="mm", name="b_ps")
                        for k in range(KT):
                            nc.tensor.matmul(
                                ps[:], w3sb[:, k, m * 128:(m + 1) * 128],
                                nrm[:, k, :], start=(k == 0), stop=(k == KT - 1))
                        nc.vector.tensor_tensor(absb[:, m, :], ps[:],
                                                asb[:, m, :], OP.mult)
                    w2sb = wr.tile([128, NH, D], BF16, tag="rws",
                                   name=f"w2sb{it}")
                    nc.sync.dma_start(w2sb[:], rw2_t[:])
                    for m in range(KT):
                        ps = ps_mm.tile([128, TB], F32, tag="mm", name="o_ps")
                        for k in range(NH):
                            nc.tensor.matmul(
                                ps[:], w2sb[:, k, m * 128:(m + 1) * 128],
                                absb[:, k, :], start=(k == 0),
                                stop=(k == NH - 1))
                        nc.vector.tensor_tensor(rs[:, m, :], ps[:], rs[:, m, :],
                                                OP.add)
                # final: out = x2 + rs
                for k in range(KT):
                    fo = tmp.tile([128, TB], F32, tag="tf32", name="fo")
                    nc.vector.tensor_tensor(fo[:], x2[:, k, :], rs[:, k, :],
                                            OP.add)
                    nc.sync.dma_start(out_ext[:, k, :], fo[:])

    nc.compile()
    return nc


def _prep_in_maps(inputs):
    f32 = np.float32
    x = np.asarray(inputs["x"], f32).reshape(NT, D)
    fcos = np.asarray(inputs["freqs_cos"], f32)
    fsin = np.asarray(inputs["freqs_sin"], f32)

    norm_cols = np.stack([
        _cw(np.asarray(inputs["attn_norm_w"], f32)),
        _cw(np.asarray(inputs["rnn_cnorm_w"], f32)),
        _cw(np.asarray(inputs["ffn_norm_w"], f32)),
        _cw(np.asarray(inputs["r_norm_w"], f32)),
        _cw(np.asarray(inputs["lg_b1"], f32)),
        _cw(np.asarray(inputs["lg_ln_w"], f32)),
        _cw(np.asarray(inputs["lg_ln_b"], f32)),
        _cw(np.asarray(inputs["lg_b2"], f32)),
    ], axis=1)  # (128, 8, KT)

    maskT = np.zeros((128, 4, TB), f32)
    ar = np.arange(TB)
    for r in range(4):
        for k in range(128):
            maskT[k, r, :] = (128 * r + k <= ar)

    shared = {
        "cos4": np.ascontiguousarray(np.tile(fcos.T, (4, 1))).astype(f32),
        "sin4": np.ascontiguousarray(np.tile(fsin.T, (4, 1))).astype(f32),
        "maskT": maskT.astype(BF),
        "norm_cols": np.ascontiguousarray(norm_cols),
        "mgb": np.asarray(inputs["mem_gate_b"], f32).reshape(1, 1),
        "wo_t": _tw(np.asarray(inputs["wo"], f32)),
        "wog_t": _tw(np.asarray(inputs["rnn_out_w"], f32)),
        "mcomp_t": _tw(np.asarray(inputs["mem_comp_w"], f32)),
        "mq_t": _tw(np.asarray(inputs["mem_q_w"], f32)),
        "mk_t": _tw(np.asarray(inputs["mem_k_w"], f32)),
        "mv_t": _tw(np.asarray(inputs["mem_v_w"], f32)),
        "mg_t": _tw(np.asarray(inputs["mem_gate_w"], f32)),
        "lgr_t": _tw(np.asarray(inputs["lg_rnn_w"], f32)),
        "lga_t": _tw(np.asarray(inputs["lg_attn_w"], f32)),
        "w1a_t": _tw(np.asarray(inputs["lg_w1"], f32)[:, :D]),
        "w1b_t": _tw(np.asarray(inputs["lg_w1"], f32)[:, D:]),
        "w2_t": _tw(np.asarray(inputs["lg_w2"], f32)),
        "rw1_t": _tw(np.asarray(inputs["r_w1"], f32)),
        "rw3_t": _tw(np.asarray(inputs["r_w3"], f32)),
        "rw2_t": _tw(np.asarray(inputs["r_w2"], f32)),
    }

    wq = np.asarray(inputs["wq"], f32)
    wk = np.asarray(inputs["wk"], f32)
    wv = np.asarray(inputs["wv"], f32)
    wgate = np.asarray(inputs["rnn_gate_w"], f32)
    wu = np.asarray(inputs["rnn_in_w"], f32)[:D, :]
    gb = np.asarray(inputs["rnn_gate_b"], f32)

    in_maps = []
    for c in range(NC):
        perm = _head_perm(2 * c) + _head_perm(2 * c + 1)
        beta = c // 4
        bm = np.full((128, 2 * M), -480.0, f32)
        bm[:, beta * M:(beta + 1) * M] = 0.0
        xb = x[c * TB:(c + 1) * TB, :]
        m = {
            "xt": np.ascontiguousarray(
                xb.T.reshape(KT, 128, TB).transpose(1, 0, 2)).astype(f32),
            "wq_t": _tw(wq[perm, :]),
            "wk_t": _tw(wk[perm, :]),
            "wv_t": _tw(wv[2 * c * HD:(2 * c + 2) * HD, :]),
            "wg_t": _tw(wgate[128 * c:128 * (c + 1), :]),
            "wu_t": _tw(wu[128 * c:128 * (c + 1), :]),
            "gate_b": np.ascontiguousarray(
                gb[128 * c:128 * (c + 1)].reshape(128, 1)).astype(f32),
            "bmask": bm,
        }
        m.update(shared)
        in_maps.append(m)
    return in_maps


def _get_program():
    if "nc" not in _PROG_CACHE:
        _PROG_CACHE["nc"] = _build_program()
    return _PROG_CACHE["nc"]


def run_kernel_internal(inputs, **run_kwargs):
    nc = _get_program()
    in_maps = _prep_in_maps(inputs)
    res = run_bass_kernel_spmd(nc, in_maps, list(range(NC)), **run_kwargs)
    out = np.empty((NT, D), np.float32)
    for c in range(NC):
        blk = np.asarray(res.results[c]["out"], np.float32)   # (128, KT, TB)
        out[c * TB:(c + 1) * TB, :] = blk.transpose(1, 0, 2).reshape(D, TB).T
    return out.reshape(B, T, D), res


def kernel(**inputs):
    out, _ = run_kernel_internal(inputs)
    return out



# revision 12
# speedup vs baseline: 1.1837x; 1.1837x over previous
"""Trainium2 Bass kernel for nn_CoeusBlockOptimized — 8-core SPMD.

Sharding: the parallel phase is feature/head-sharded (core c owns attention
heads (2c, 2c+1) and recurrence D-shard [128c, 128c+128) for ALL tokens); the
tail is token-sharded (core c owns flattened token block [512c, 512c+512)).
Cross-core traffic: one bf16 AllGather of h^T (1 MB/rank) and one bf16
AllToAll carrying attention output + recurrence state (2 MB/rank).

All activations are kept transposed (feature rows on partitions, tokens on
the free axis); per-token reductions (rms/layernorm/softmax denominators) use
ones-vector matmuls on the PE plus a K=1 broadcast matmul back to 128
partitions.  Matmuls run in bf16 (fp32 PSUM accumulation); the sequential
recurrence uses the VectorE tensor_tensor_scan instruction in fp32.

Schedule notes: the AllGather is triggered as early as possible (only the
xt load + rms precede it); all weight loads are deferred behind it.  The
recurrence runs before attention (dense matmuls, warms the PE).  Reasoning
weights are streamed in 4 chunks with a 3-deep rotation so the PE never
waits on a whole-tensor DMA.  Row reciprocals use the fast custom-DVE
approximation (the exact InstReciprocal is ~6.5ns/elem on one partition).
"""
import sys
import os

for _p in ("/opt/trn_rl_repo", "/root/.axon_site/_ro/trn_rl_repo"):
    if os.path.isdir(_p) and _p not in sys.path:
        sys.path.insert(0, _p)

import numpy as np
import ml_dtypes

import concourse.bass as bass
import concourse.tile as tile
from concourse import mybir, bacc
from concourse.bass_utils import run_bass_kernel_spmd
from concourse.masks import make_identity

BF = ml_dtypes.bfloat16
F32 = mybir.dt.float32
BF16 = mybir.dt.bfloat16
AF = mybir.ActivationFunctionType
OP = mybir.AluOpType

NC = 8
B, T, D = 2, 2048, 1024
H, HD, HID, M = 16, 64, 3072, 256
FREQ = 8
TB = 512            # tokens per core block
KT = D // 128       # 8 d-tiles
NH = HID // 128     # 24
NT = B * T          # 4096 tokens
NCH = 4             # weight streaming chunks in the reasoning block
M1 = NH // NCH      # m-tiles per w1/w3 chunk
M2 = KT // NCH      # m-tiles per w2 chunk

_PROG_CACHE = {}


def _tw(w):
    """(Mout, Kin) weight -> (128, Kin/128, Mout) bf16 lhsT-tile layout."""
    k, m = w.shape[1], w.shape[0]
    assert k % 128 == 0
    return np.ascontiguousarray(
        w.T.reshape(k // 128, 128, m).transpose(1, 0, 2)).astype(BF)


def _tw_chunks(w, nch=NCH):
    """_tw layout split into nch contiguous column chunks:
    (128, Kin/128, Mout) -> (128, nch, Kin/128, Mout/nch)."""
    t = _tw(w)
    mc = t.shape[2] // nch
    return np.ascontiguousarray(
        t.reshape(128, t.shape[1], nch, mc).transpose(0, 2, 1, 3))


def _cw(v):
    """(1024,) vector -> (128, 8) fp32 per-partition column layout."""
    return np.ascontiguousarray(v.reshape(-1, 128).T).astype(np.float32)


def _head_perm(h):
    base = h * HD
    return [base + i for i in range(0, HD, 2)] + [base + i for i in range(1, HD, 2)]


def _build_program():
    nc = bacc.Bacc("TRN2", target_bir_lowering=False, debug=False, num_devices=NC)

    def din(name, shape, dt):
        return nc.dram_tensor(name, list(shape), dt, kind="ExternalInput")

    # per-core inputs
    xt = din("xt", (128, KT, TB), F32)
    wq_t = din("wq_t", (128, KT, 128), BF16)
    wk_t = din("wk_t", (128, KT, 128), BF16)
    wv_t = din("wv_t", (128, KT, 128), BF16)
    wg_t = din("wg_t", (128, KT, 128), BF16)
    wu_t = din("wu_t", (128, KT, 128), BF16)
    gate_b = din("gate_b", (128, 1), F32)
    bmask = din("bmask", (128, 2 * M), F32)
    # shared inputs
    cos4 = din("cos4", (128, T), BF16)
    sin4 = din("sin4", (128, T), BF16)
    maskT_in = din("maskT", (128, 4, TB), BF16)
    norm_cols = din("norm_cols", (128, 8, KT), F32)   # packed norm/bias columns
    mgb = din("mgb", (1, 1), F32)
    wo_t = din("wo_t", (128, KT, D), BF16)
    wog_t = din("wog_t", (128, KT, D), BF16)
    mcomp_t = din("mcomp_t", (128, KT, M), BF16)
    mq_t = din("mq_t", (128, KT, M), BF16)
    mk_t = din("mk_t", (128, 2, M), BF16)
    mv_t = din("mv_t", (128, 2, D), BF16)
    mg_t = din("mg_t", (128, KT, 1), BF16)
    lgr_t = din("lgr_t", (128, KT, D), BF16)
    lga_t = din("lga_t", (128, KT, D), BF16)
    w1a_t = din("w1a_t", (128, KT, D), BF16)
    w1b_t = din("w1b_t", (128, KT, D), BF16)
    w2_t = din("w2_t", (128, KT, D), BF16)
    rw1_t = din("rw1_t", (128, NCH, KT, HID // NCH), BF16)
    rw3_t = din("rw3_t", (128, NCH, KT, HID // NCH), BF16)
    rw2_t = din("rw2_t", (128, NCH, NH, D // NCH), BF16)

    out_ext = nc.dram_tensor("out", [128, KT, TB], F32, kind="ExternalOutput")
    DEBUG = bool(int(os.environ.get("KERNEL_DEBUG_DUMPS", "0")))
    if DEBUG:
        dbg = {nm: nc.dram_tensor(nm, [128, KT, TB], F32, kind="ExternalOutput")
               for nm in ("d_ht0", "d_hstT", "d_atnT", "d_ol", "d_og", "d_zt",
                          "d_x2a", "d_x2b", "d_nrm0")}

    # norm_cols packing order
    NCOL = {"anorm": 0, "cnorm": 1, "ffn": 2, "rn": 3, "b1": 4, "lnw": 5,
            "lnb": 6, "b2": 7}

    with tile.TileContext(nc) as tc:
        with tc.tile_pool(name="dram", bufs=1, space="DRAM") as dram, \
             tc.tile_pool(name="const", bufs=1) as const, \
             tc.tile_pool(name="resid", bufs=1) as resid, \
             tc.tile_pool(name="tmp", bufs=3) as tmp, \
             tc.tile_pool(name="rows", bufs=4) as rows, \
             tc.tile_pool(name="ps_mm", bufs=4, space="PSUM") as ps_mm, \
             tc.tile_pool(name="ps_bc", bufs=2, space="PSUM") as ps_bc, \
             tc.tile_pool(name="ps_row", bufs=2, space="PSUM") as ps_row:

            # ---- DRAM comm buffers ----
            ag_in = dram.tile([KT, 128, TB], BF16)
            ag_out = dram.tile([NC, KT, 128, TB], BF16, addr_space="Shared")
            a2a_in = dram.tile([NC, 2, 128, TB], BF16)
            a2a_out = dram.tile([NC, 2, 128, TB], BF16)

            # ---- tiny constants needed by phase 0 ----
            ones_col = const.tile([128, 1], BF16)
            ones_row = const.tile([1, 128], BF16)
            ident = const.tile([128, 128], BF16)
            nc.any.memset(ones_col[:], 1.0)
            nc.any.memset(ones_row[:], 1.0)
            make_identity(nc, ident[:])
            ncols = const.tile([128, 8, KT], F32)
            eps6_sb = const.tile([1, 1], F32)
            eps5_sb = const.tile([1, 1], F32)
            nc.any.memset(eps6_sb[:], 1e-6)
            nc.any.memset(eps5_sb[:], 1e-5)
            nc.sync.dma_start(ncols[:], norm_cols[:])

            def ncol(nm, k):
                return ncols[:, NCOL[nm], k:k + 1]

            # ---- long-lived activations ----
            x2 = resid.tile([128, KT, TB], F32)
            atnT = resid.tile([128, KT, TB], BF16)
            hstT = resid.tile([128, KT, TB], BF16)

            # ---------- shared helpers ----------
            def rms_rsqrt_row(src_getter, eps_ap, nk=KT, width=TB):
                ss = ps_row.tile([1, width], F32, tag="psrow", name="ss")
                for k in range(nk):
                    sq = tmp.tile([128, width], BF16, tag="tb16", name="sq")
                    nc.vector.tensor_tensor(sq[:], src_getter(k), src_getter(k),
                                            OP.mult)
                    nc.tensor.matmul(ss[:], ones_col[:], sq[:],
                                     start=(k == 0), stop=(k == nk - 1))
                ms = rows.tile([1, width], F32, tag="row32", name="ms")
                nc.scalar.activation(ms[:], ss[:], AF.Identity,
                                     bias=eps_ap[0:1, 0:1], scale=1.0 / (nk * 128))
                rc = rows.tile([1, width], F32, tag="row32", name="rc")
                nc.vector.reciprocal_approx_fast(rc[:], ms[:])
                rs_row = rows.tile([1, width], BF16, tag="row16", name="rsr")
                nc.scalar.activation(rs_row[:], rc[:], AF.Sqrt)
                return rs_row

            def bcast_row(row_bf, width=TB, np_=128):
                bc = ps_bc.tile([np_, width], F32, tag="bc", name="bc")
                nc.tensor.matmul(bc[:], ones_row[0:1, 0:np_], row_bf[:])
                return bc

            with tc.tile_pool(name="ep0", bufs=1) as ep0:
                qtm = ep0.tile([128, 2, TB], BF16)
                grow = ep0.tile([1, TB], BF16)
                gbc = ep0.tile([128, TB], BF16)

                # ============== phase 0: local h^T + AllGather ==============
                with tc.tile_pool(name="htloc", bufs=1) as htloc_pool:
                    ht_loc = htloc_pool.tile([128, KT, TB], BF16)
                    with tc.tile_pool(name="ph0", bufs=1) as ph0:
                        xt_sb = ph0.tile([128, KT, TB], F32)
                        nc.sync.dma_start(xt_sb[:], xt[:])
                        rsq = rms_rsqrt_row(lambda k: xt_sb[:, k, :], eps6_sb)
                        bc = bcast_row(rsq)
                        for k in range(KT):
                            nc.vector.scalar_tensor_tensor(
                                ht_loc[:, k, :], xt_sb[:, k, :], ncol("anorm", k),
                                bc[:], OP.mult, OP.mult)
                            nc.sync.dma_start(ag_in[k], ht_loc[:, k, :])
                    nc.gpsimd.collective_compute(
                        "AllGather", OP.bypass,
                        replica_groups=[list(range(NC))],
                        ins=[ag_in.opt()], outs=[ag_out.opt()])

                    # ---- deferred constant loads (overlap the AllGather) ----
                    cos_sb = const.tile([128, T], BF16)
                    sin_sb = const.tile([128, T], BF16)
                    gate_b_sb = const.tile([128, 1], F32)
                    mgb_sb = const.tile([1, 1], F32)
                    mask_sb = const.tile([128, 4, TB], BF16)
                    bmask_sb = const.tile([128, 2 * M], F32)
                    nc.sync.dma_start(mgb_sb[:], mgb[:])
                    wq_sb = const.tile([128, KT, 128], BF16)
                    wk_sb = const.tile([128, KT, 128], BF16)
                    wv_sb = const.tile([128, KT, 128], BF16)
                    wg_sb = const.tile([128, KT, 128], BF16)
                    wu_sb = const.tile([128, KT, 128], BF16)
                    mcomp_sb = const.tile([128, KT, M], BF16)
                    mq_sb = const.tile([128, KT, M], BF16)
                    mk_sb = const.tile([128, 2, M], BF16)
                    mv_sb = const.tile([128, 2, D], BF16)
                    mg_sb = const.tile([128, KT, 1], BF16)
                    for sb, t_in in ((mq_sb, mq_t), (mg_sb, mg_t), (wg_sb, wg_t),
                                     (wu_sb, wu_t), (wq_sb, wq_t), (wk_sb, wk_t),
                                     (wv_sb, wv_t), (mcomp_sb, mcomp_t),
                                     (mk_sb, mk_t), (mv_sb, mv_t)):
                        nc.sync.dma_start(sb[:], t_in[:])
                    nc.sync.dma_start(gate_b_sb[:], gate_b[:])
                    nc.sync.dma_start(cos_sb[:], cos4[:])
                    nc.sync.dma_start(sin_sb[:], sin4[:])
                    nc.sync.dma_start(mask_sb[:], maskT_in[:])
                    nc.sync.dma_start(bmask_sb[:], bmask[:])

                    # episodic local-only pieces run during the AllGather
                    for mi in range(2):
                        ps = ps_mm.tile([128, TB], F32, tag="mm", name="q_ps")
                        for k in range(KT):
                            nc.tensor.matmul(
                                ps[:],
                                mq_sb[:, k, mi * 128:(mi + 1) * 128],
                                ht_loc[:, k, :], start=(k == 0),
                                stop=(k == KT - 1))
                        nc.scalar.copy(qtm[:, mi, :], ps[:])
                    psg2 = ps_row.tile([1, TB], F32, tag="psrow", name="g_psr")
                    for k in range(KT):
                        nc.tensor.matmul(psg2[:], mg_sb[:, k, :],
                                         ht_loc[:, k, :],
                                         start=(k == 0), stop=(k == KT - 1))
                    nc.scalar.activation(grow[:], psg2[:], AF.Sigmoid,
                                         bias=mgb_sb[0:1, 0:1])
                    gb = bcast_row(grow)
                    nc.scalar.copy(gbc[:], gb[:])

                with tc.tile_pool(name="ht", bufs=1) as ht_pool:
                    ht = ht_pool.tile([128, KT, NC, TB], BF16)
                    for k in range(KT):
                        nc.sync.dma_start(
                            ht[:, k, :, :],
                            ag_out[:, k, :, :].rearrange("blk p t -> p blk t"))
                    if DEBUG:
                        nc.gpsimd.dma_start(dbg["d_ht0"][:], ht[:, :, 0, :])

                    # ========= phase 1: recurrence (dense, runs first) =========
                    with tc.tile_pool(name="scan", bufs=1) as scan_pool:
                        gate_sb = scan_pool.tile([128, NC, TB], F32)
                        u_sb = scan_pool.tile([128, NC, TB], F32)
                        hst_sb = scan_pool.tile([128, NC * TB], BF16)
                        for blk in range(NC):
                            psg = ps_mm.tile([128, TB], F32, tag="mm",
                                             name="g_ps")
                            for k in range(KT):
                                nc.tensor.matmul(psg[:], wg_sb[:, k, :],
                                                 ht[:, k, blk, :],
                                                 start=(k == 0),
                                                 stop=(k == KT - 1))
                            nc.scalar.activation(gate_sb[:, blk, :], psg[:],
                                                 AF.Sigmoid,
                                                 bias=gate_b_sb[:])
                            psu = ps_mm.tile([128, TB], F32, tag="mm",
                                             name="u_ps")
                            for k in range(KT):
                                nc.tensor.matmul(psu[:], wu_sb[:, k, :],
                                                 ht[:, k, blk, :],
                                                 start=(k == 0),
                                                 stop=(k == KT - 1))
                            nc.scalar.activation(u_sb[:, blk, :], psu[:],
                                                 AF.Silu)
                        g2d = gate_sb[:].rearrange("p a b -> p (a b)")
                        u2d = u_sb[:].rearrange("p a b -> p (a b)")
                        for b in range(B):
                            cols = slice(b * T, (b + 1) * T)
                            nc.vector.tensor_tensor_scan(
                                hst_sb[:, cols], g2d[:, cols], u2d[:, cols],
                                0.0, OP.mult, OP.add)
                        for j in range(NC):
                            nc.sync.dma_start(
                                a2a_in[j, 1], hst_sb[:, j * TB:(j + 1) * TB])

                    # ==================== phase 2: attention ====================
                    with tc.tile_pool(name="attn", bufs=1) as attn:
                        for b in range(B):
                            qT = attn.tile([128, T], BF16, tag="qT",
                                           name=f"qT{b}")
                            kTt = attn.tile([128, T], BF16, tag="kT",
                                            name=f"kT{b}")
                            vn = attn.tile([128, 16, 130], BF16, tag="vn",
                                           name=f"vn{b}")
                            nc.any.memset(vn[:, :, 64:65], 1.0)
                            nc.any.memset(vn[:, :, 129:130], 1.0)

                            for dst, wsb in ((qT, wq_sb), (kTt, wk_sb)):
                                for nb in range(4):
                                    cols = slice(nb * TB, (nb + 1) * TB)
                                    ps = ps_mm.tile([128, TB], F32, tag="mm",
                                                    name="qk_ps")
                                    for k in range(KT):
                                        nc.tensor.matmul(
                                            ps[:], wsb[:, k, :],
                                            ht[:, k, 4 * b + nb, :],
                                            start=(k == 0),
                                            stop=(k == KT - 1))
                                    m1 = tmp.tile([128, TB], F32, tag="tf32",
                                                  name="rot1")
                                    m2 = ps_bc.tile([128, TB], F32, tag="bc",
                                                    name="rot2")
                                    nc.vector.tensor_tensor(
                                        m1[:], ps[:], cos_sb[:, cols], OP.mult)
                                    nc.vector.tensor_tensor(
                                        m2[:], ps[:], sin_sb[:, cols], OP.mult)
                                    for h2 in range(2):
                                        be = 64 * h2
                                        nc.vector.tensor_tensor(
                                            dst[be:be + 32, cols],
                                            m1[be:be + 32, :],
                                            m2[be + 32:be + 64, :],
                                            OP.subtract)
                                        nc.vector.tensor_tensor(
                                            dst[be + 32:be + 64, cols],
                                            m1[be + 32:be + 64, :],
                                            m2[be:be + 32, :], OP.add)

                            for tt in range(16):
                                nb, sub = tt // 4, tt % 4
                                ps = ps_mm.tile([128, 128], F32, tag="mm",
                                                name="v_ps")
                                for k in range(KT):
                                    nc.tensor.matmul(
                                        ps[:],
                                        ht[:, k, 4 * b + nb,
                                           sub * 128:(sub + 1) * 128],
                                        wv_sb[:, k, :],
                                        start=(k == 0), stop=(k == KT - 1))
                                nc.scalar.copy(vn[:, tt, 0:64], ps[:, 0:64])
                                nc.scalar.copy(vn[:, tt, 65:129],
                                               ps[:, 64:128])

                            for h2 in range(2):
                                be = 64 * h2
                                for qb in range(4):
                                    qcols = slice(qb * TB, (qb + 1) * TB)
                                    nlive = 4 * qb + 4
                                    pt = attn.tile([128, 16, TB], BF16,
                                                   tag="pt", name="pt",
                                                   bufs=2)
                                    for ki in range(nlive):
                                        sps = ps_mm.tile([128, TB], F32,
                                                         tag="mm", name="s_ps")
                                        nc.tensor.matmul(
                                            sps[:],
                                            kTt[be:be + 64,
                                                ki * 128:(ki + 1) * 128],
                                            qT[be:be + 64, qcols],
                                            start=True, stop=True)
                                        nc.scalar.activation(
                                            pt[:, ki, :], sps[:], AF.Exp,
                                            scale=0.125)
                                        if ki >= 4 * qb:
                                            nc.vector.tensor_tensor(
                                                pt[:, ki, :], pt[:, ki, :],
                                                mask_sb[:, ki - 4 * qb, :],
                                                OP.mult)
                                    av = ps_mm.tile([65, TB], F32, tag="mm",
                                                    name="av_ps")
                                    for ki in range(nlive):
                                        nc.tensor.matmul(
                                            av[:],
                                            vn[:, ki, 65 * h2:65 * h2 + 65],
                                            pt[:, ki, :], start=(ki == 0),
                                            stop=(ki == nlive - 1))
                                    dnr = rows.tile([1, TB], F32, tag="row32",
                                                    name="dnr")
                                    nc.scalar.copy(dnr[:], av[64:65, :])
                                    rd = rows.tile([1, TB], F32, tag="row32",
                                                   name="rd")
                                    nc.vector.reciprocal_approx_fast(
                                        rd[:], dnr[:])
                                    rdb = rows.tile([1, TB], BF16,
                                                    tag="row16", name="rdb")
                                    nc.scalar.copy(rdb[:], rd[:])
                                    bcd = bcast_row(rdb, np_=64)
                                    bsb = tmp.tile([64, TB], BF16, tag="t64",
                                                   name="bsb", bufs=4)
                                    nc.scalar.copy(bsb[:], bcd[:])
                                    an_t = tmp.tile([64, TB], BF16,
                                                    tag="t64", name="an_t",
                                                    bufs=4)
                                    nc.vector.tensor_tensor(
                                        an_t[:], av[0:64, :], bsb[:], OP.mult)
                                    nc.sync.dma_start(
                                        a2a_in[4 * b + qb, 0,
                                               be:be + 64, :], an_t[:])

                    nc.gpsimd.collective_compute(
                        "AllToAll", OP.bypass,
                        replica_groups=[list(range(NC))],
                        ins=[a2a_in.opt()], outs=[a2a_out.opt()])
                    # tail inputs gathered as soon as the A2A lands
                    nc.sync.dma_start(atnT[:],
                                      a2a_out[:, 0].rearrange("i p t -> p i t"))
                    nc.sync.dma_start(hstT[:],
                                      a2a_out[:, 1].rearrange("i p t -> p i t"))
                    if DEBUG:
                        nc.gpsimd.dma_start(dbg["d_atnT"][:], atnT[:])
                        nc.gpsimd.dma_start(dbg["d_hstT"][:], hstT[:])

                    # ========= phase 3: episodic (overlaps the A2A) =========
                    with tc.tile_pool(name="ep", bufs=1) as ep:
                        memk = ep.tile([128, 2, 2 * M], BF16)
                        ktm = ep.tile([128, 2, 2 * M], BF16)
                        a_sb = ep.tile([128, 4, 2 * M], BF16)
                        at_sb = ep.tile([128, 4, TB], BF16)
                        sn_sb = ep.tile([128, 4, D], BF16)
                        mo_sb = ep.tile([128, KT, TB], BF16)
                        moc = ep.tile([128, 2, TB], BF16)

                        for mi in range(2):
                            ps = ps_mm.tile([128, 2 * M], F32, tag="mm",
                                            name="mk_ps")
                            for k in range(KT):
                                nc.tensor.matmul(
                                    ps[:],
                                    mcomp_sb[:, k, mi * 128:(mi + 1) * 128],
                                    ht[:, k, :, 0:TB:FREQ],
                                    start=(k == 0), stop=(k == KT - 1))
                            nc.scalar.copy(memk[:, mi, :], ps[:])
                        for mo in range(2):
                            ps = ps_mm.tile([128, 2 * M], F32, tag="mm",
                                            name="kt_ps")
                            for mi in range(2):
                                nc.tensor.matmul(
                                    ps[:],
                                    mk_sb[:, mi, mo * 128:(mo + 1) * 128],
                                    memk[:, mi, :], start=(mi == 0),
                                    stop=(mi == 1))
                            nc.scalar.copy(ktm[:, mo, :], ps[:])
                        for tt in range(4):
                            ps = ps_mm.tile([128, 2 * M], F32, tag="mm",
                                            name="sc_ps")
                            for mi in range(2):
                                nc.tensor.matmul(
                                    ps[:],
                                    qtm[:, mi, tt * 128:(tt + 1) * 128],
                                    ktm[:, mi, :], start=(mi == 0),
                                    stop=(mi == 1))
                            sm = tmp.tile([128, 2 * M], F32, tag="tf32",
                                          name="sm")
                            nc.vector.tensor_tensor(sm[:], ps[:], bmask_sb[:],
                                                    OP.add)
                            den = rows.tile([128, 1], F32, tag="den",
                                            name="den")
                            nc.scalar.activation(a_sb[:, tt, :], sm[:],
                                                 AF.Exp, scale=1.0 / 16.0,
                                                 accum_out=den[:])
                            rden = rows.tile([128, 1], F32, tag="den",
                                             name="rden")
                            nc.vector.reciprocal(rden[:], den[:])
                            nc.vector.tensor_scalar_mul(
                                a_sb[:, tt, :], a_sb[:, tt, :], rden[:])
                        for st in range(4):
                            for k in range(KT):
                                pst = ps_mm.tile([128, 128], BF16, tag="mm",
                                                 name="tr_ps")
                                nc.tensor.transpose(
                                    pst[:],
                                    ht[:, k, 2 * st:2 * st + 2, 0:TB:FREQ],
                                    ident[:])
                                nc.scalar.copy(
                                    sn_sb[:, st, k * 128:(k + 1) * 128],
                                    pst[:])
                            for tt in range(4):
                                pst = ps_mm.tile([128, 128], BF16, tag="mm",
                                                 name="tr2_ps")
                                nc.tensor.transpose(
                                    pst[:],
                                    a_sb[:, tt, st * 128:(st + 1) * 128],
                                    ident[:])
                                nc.scalar.copy(
                                    at_sb[:, st, tt * 128:(tt + 1) * 128],
                                    pst[:])
                        for dm in range(KT):
                            ps = ps_mm.tile([128, TB], F32, tag="mm",
                                            name="mo_ps")
                            for st in range(4):
                                nc.tensor.matmul(
                                    ps[:],
                                    sn_sb[:, st, dm * 128:(dm + 1) * 128],
                                    at_sb[:, st, :], start=(st == 0),
                                    stop=(st == 3))
                            nc.scalar.copy(mo_sb[:, dm, :], ps[:])
                        for mi in range(2):
                            ps = ps_mm.tile([128, TB], F32, tag="mm",
                                            name="moc_ps")
                            for k in range(KT):
                                nc.tensor.matmul(
                                    ps[:],
                                    mcomp_sb[:, k, mi * 128:(mi + 1) * 128],
                                    mo_sb[:, k, :], start=(k == 0),
                                    stop=(k == KT - 1))
                            nc.scalar.copy(moc[:, mi, :], ps[:])
                        # x2 = x + out_episodic (residual base, pre-tail)
                        for dm in range(KT):
                            ps = ps_mm.tile([128, TB], F32, tag="mm",
                                            name="mv_ps")
                            for mi in range(2):
                                nc.tensor.matmul(
                                    ps[:],
                                    mv_sb[:, mi, dm * 128:(dm + 1) * 128],
                                    moc[:, mi, :], start=(mi == 0),
                                    stop=(mi == 1))
                            e1 = tmp.tile([128, TB], BF16, tag="tb16",
                                          name="e1")
                            nc.vector.tensor_tensor(e1[:], ps[:], gbc[:],
                                                    OP.mult)
                            xrk = tmp.tile([128, TB], F32, tag="tf32",
                                           name="xrk")
                            nc.sync.dma_start(xrk[:], xt[:, dm, :])
                            nc.vector.tensor_tensor(x2[:, dm, :], xrk[:],
                                                    e1[:], OP.add)
                        if DEBUG:
                            nc.gpsimd.dma_start(dbg["d_x2a"][:], x2[:])

            # ================= phase 4: tail (token-parallel) =================
            with tc.tile_pool(name="tlg", bufs=1) as tlg, \
                 tc.tile_pool(name="wt", bufs=2) as wt:

                def mm_chain(w_dram, rhs_fn, evict, nk=KT, nm=KT):
                    wsb = wt.tile([128, nk, D], BF16, tag="w", name="wstream")
                    nc.sync.dma_start(wsb[:], w_dram[:])
                    for m in range(nm):
                        ps = ps_mm.tile([128, TB], F32, tag="mm", name="c_ps")
                        for k in range(nk):
                            nc.tensor.matmul(ps[:],
                                             wsb[:, k, m * 128:(m + 1) * 128],
                                             rhs_fn(k), start=(k == 0),
                                             stop=(k == nk - 1))
                        evict(m, ps)

                ol = tlg.tile([128, KT, TB], BF16)
                mm_chain(wo_t, lambda k: atnT[:, k, :],
                         lambda m, ps: nc.scalar.copy(ol[:, m, :], ps[:]))
                if DEBUG:
                    nc.gpsimd.dma_start(dbg["d_ol"][:], ol[:])
                hatt = tlg.tile([128, KT, TB], BF16, tag="hio",
                                name="hatt", bufs=2)
                mm_chain(lga_t, lambda k: ol[:, k, :],
                         lambda m, ps: nc.scalar.copy(hatt[:, m, :], ps[:]))

                nT = tlg.tile([128, KT, TB], BF16, tag="nT", name="nT")
                rsq2 = rms_rsqrt_row(lambda k: hstT[:, k, :], eps6_sb)
                bc2 = bcast_row(rsq2)
                for k in range(KT):
                    nc.vector.scalar_tensor_tensor(
                        nT[:, k, :], hstT[:, k, :], ncol("cnorm", k), bc2[:],
                        OP.mult, OP.mult)
                og = tlg.tile([128, KT, TB], BF16)
                mm_chain(wog_t, lambda k: nT[:, k, :],
                         lambda m, ps: nc.scalar.copy(og[:, m, :], ps[:]))
                if DEBUG:
                    nc.gpsimd.dma_start(dbg["d_og"][:], og[:])
                hrnn = tlg.tile([128, KT, TB], BF16, tag="hio",
                                name="hrnn", bufs=2)
                mm_chain(lgr_t, lambda k: og[:, k, :],
                         lambda m, ps: nc.scalar.copy(hrnn[:, m, :], ps[:]))

                t1b = tlg.tile([128, KT, TB], BF16, tag="t1b", name="t1b")
                w1a_sb = wt.tile([128, KT, D], BF16, tag="w", name="w1a_sb")
                w1b_sb = wt.tile([128, KT, D], BF16, tag="w", name="w1b_sb")
                nc.sync.dma_start(w1b_sb[:], w1b_t[:])
                nc.sync.dma_start(w1a_sb[:], w1a_t[:])
                for m in range(KT):
                    ps = ps_mm.tile([128, TB], F32, tag="mm", name="t1_ps")
                    for k in range(KT):
                        nc.tensor.matmul(ps[:],
                                         w1b_sb[:, k, m * 128:(m + 1) * 128],
                                         hatt[:, k, :], start=(k == 0),
                                         stop=False)
                    for k in range(KT):
                        nc.tensor.matmul(ps[:],
                                         w1a_sb[:, k, m * 128:(m + 1) * 128],
                                         hrnn[:, k, :], start=False,
                                         stop=(k == KT - 1))
                    nc.scalar.activation(t1b[:, m, :], ps[:], AF.Identity,
                                         bias=ncol("b1", m))

                # layernorm stats (mean via ones-matmul; fast reciprocal)
                ssum = ps_row.tile([1, TB], F32, tag="psrow", name="ssum")
                for k in range(KT):
                    nc.tensor.matmul(ssum[:], ones_col[:], t1b[:, k, :],
                                     start=(k == 0), stop=(k == KT - 1))
                ssq = ps_row.tile([1, TB], F32, tag="psrow", name="ssq")
                for k in range(KT):
                    sq = tmp.tile([128, TB], BF16, tag="tb16", name="sq2")
                    nc.vector.tensor_tensor(sq[:], t1b[:, k, :], t1b[:, k, :],
                                            OP.mult)
                    nc.tensor.matmul(ssq[:], ones_col[:], sq[:],
                                     start=(k == 0), stop=(k == KT - 1))
                mu = rows.tile([1, TB], F32, tag="row32", name="mu")
                nc.scalar.activation(mu[:], ssum[:], AF.Identity, scale=1.0 / D)
                mub = rows.tile([1, TB], BF16, tag="row16", name="mub")
                nc.scalar.copy(mub[:], mu[:])
                mu2 = rows.tile([1, TB], F32, tag="row32", name="mu2")
                nc.vector.tensor_tensor(mu2[:], mu[:], mu[:], OP.mult)
                ex2 = rows.tile([1, TB], F32, tag="row32", name="ex2")
                nc.scalar.activation(ex2[:], ssq[:], AF.Identity, scale=1.0 / D)
                varr = rows.tile([1, TB], F32, tag="row32", name="varr")
                nc.vector.tensor_tensor(varr[:], ex2[:], mu2[:], OP.subtract)
                vre = rows.tile([1, TB], F32, tag="row32", name="vre")
                nc.scalar.activation(vre[:], varr[:], AF.Identity,
                                     bias=eps5_sb[0:1, 0:1])
                rcv = rows.tile([1, TB], F32, tag="row32", name="rcv")
                nc.vector.reciprocal_approx_fast(rcv[:], vre[:])
                rsl = rows.tile([1, TB], BF16, tag="row16", name="rsl")
                nc.scalar.activation(rsl[:], rcv[:], AF.Sqrt)
                bc_mu = bcast_row(mub)
                bc_rs = bcast_row(rsl)
                zt = tlg.tile([128, KT, TB], BF16, tag="zt", name="zt")
                for k in range(KT):
                    d1 = tmp.tile([128, TB], F32, tag="tf32", name="d1")
                    nc.vector.tensor_tensor(d1[:], t1b[:, k, :], bc_mu[:],
                                            OP.subtract)
                    d2 = tmp.tile([128, TB], F32, tag="tf32", name="d2")
                    nc.vector.tensor_tensor(d2[:], d1[:], bc_rs[:], OP.mult)
                    nc.scalar.activation(zt[:, k, :], d2[:], AF.Silu,
                                         bias=ncol("lnb", k),
                                         scale=ncol("lnw", k))
                if DEBUG:
                    nc.gpsimd.dma_start(dbg["d_zt"][:], zt[:])
                g2t = tlg.tile([128, KT, TB], BF16)
                mm_chain(w2_t, lambda k: zt[:, k, :],
                         lambda m, ps: nc.scalar.activation(
                             g2t[:, m, :], ps[:], AF.Sigmoid, bias=ncol("b2", m)))

                for k in range(KT):
                    mx1 = tmp.tile([128, TB], F32, tag="tf32", name="mx1")
                    nc.vector.tensor_tensor(mx1[:], ol[:, k, :], og[:, k, :],
                                            OP.subtract)
                    mx2 = tmp.tile([128, TB], F32, tag="tf32", name="mx2")
                    nc.vector.tensor_tensor(mx2[:], g2t[:, k, :], mx1[:],
                                            OP.mult)
                    mx3 = tmp.tile([128, TB], F32, tag="tf32", name="mx3")
                    nc.vector.tensor_tensor(mx3[:], og[:, k, :], mx2[:], OP.add)
                    nc.vector.tensor_tensor(x2[:, k, :], x2[:, k, :], mx3[:],
                                            OP.add)

            if DEBUG:
                nc.gpsimd.dma_start(dbg["d_x2b"][:], x2[:])
            # ================= phase 5: reasoning (SwiGLU x2) =================
            with tc.tile_pool(name="trs", bufs=1) as trs, \
                 tc.tile_pool(name="wr", bufs=1) as wr:
                rs = trs.tile([128, KT, TB], F32)
                rsq3 = rms_rsqrt_row(lambda k: x2[:, k, :], eps6_sb)
                bc3 = bcast_row(rsq3)
                for k in range(KT):
                    nc.vector.scalar_tensor_tensor(
                        rs[:, k, :], x2[:, k, :], ncol("ffn", k), bc3[:],
                        OP.mult, OP.mult)
                for it in range(2):
                    nrm = trs.tile([128, KT, TB], BF16, tag="nrm",
                                   name=f"nrm{it}")
                    rsq4 = rms_rsqrt_row(lambda k: rs[:, k, :], eps6_sb)
                    bc4 = bcast_row(rsq4)
                    for k in range(KT):
                        nc.vector.scalar_tensor_tensor(
                            nrm[:, k, :], rs[:, k, :], ncol("rn", k), bc4[:],
                            OP.mult, OP.mult)
                    if DEBUG and it == 0:
                        nc.gpsimd.dma_start(dbg["d_nrm0"][:], nrm[:])
                    asb = trs.tile([128, NH, TB], BF16, tag="asb",
                                   name=f"asb{it}")
                    for c in range(NCH):
                        w1c = wr.tile([128, KT, HID // NCH], BF16, tag="rwc",
                                      name="w1c", bufs=3)
                        nc.sync.dma_start(w1c[:], rw1_t[:, c])
                        for m6 in range(M1):
                            m = c * M1 + m6
                            ps = ps_mm.tile([128, TB], F32, tag="mm",
                                            name="a_ps")
                            for k in range(KT):
                                nc.tensor.matmul(
                                    ps[:], w1c[:, k, m6 * 128:(m6 + 1) * 128],
                                    nrm[:, k, :], start=(k == 0),
                                    stop=(k == KT - 1))
                            nc.scalar.activation(asb[:, m, :], ps[:], AF.Silu)
                    absb = trs.tile([128, NH, TB], BF16, tag="absb",
                                    name=f"absb{it}")
                    for c in range(NCH):
                        w3c = wr.tile([128, KT, HID // NCH], BF16, tag="rwc",
                                      name="w3c", bufs=3)
                        nc.sync.dma_start(w3c[:], rw3_t[:, c])
                        for m6 in range(M1):
                            m = c * M1 + m6
                            ps = ps_mm.tile([128, TB], F32, tag="mm",
                                            name="b_ps")
                            for k in range(KT):
                                nc.tensor.matmul(
                                    ps[:], w3c[:, k, m6 * 128:(m6 + 1) * 128],
                                    nrm[:, k, :], start=(k == 0),
                                    stop=(k == KT - 1))
                            nc.vector.tensor_tensor(absb[:, m, :], ps[:],
                                                    asb[:, m, :], OP.mult)
                    for c in range(NCH):
                        w2c = wr.tile([128, NH, D // NCH], BF16, tag="rwc",
                                      name="w2c", bufs=3)
                        nc.sync.dma_start(w2c[:], rw2_t[:, c])
                        for m2 in range(M2):
                            m = c * M2 + m2
                            ps = ps_mm.tile([128, TB], F32, tag="mm",
                                            name="o_ps")
                            for k in range(NH):
                                nc.tensor.matmul(
                                    ps[:], w2c[:, k, m2 * 128:(m2 + 1) * 128],
                                    absb[:, k, :], start=(k == 0),
                                    stop=(k == NH - 1))
                            nc.vector.tensor_tensor(rs[:, m, :], ps[:],
                                                    rs[:, m, :], OP.add)
                # final: out = x2 + rs
                for k in range(KT):
                    fo = tmp.tile([128, TB], F32, tag="tf32", name="fo")
                    nc.vector.tensor_tensor(fo[:], x2[:, k, :], rs[:, k, :],
                                            OP.add)
                    nc.sync.dma_start(out_ext[:, k, :], fo[:])

    nc.compile()
    return nc


def _prep_in_maps(inputs):
    f32 = np.float32
    x = np.asarray(inputs["x"], f32).reshape(NT, D)
    fcos = np.asarray(inputs["freqs_cos"], f32)
    fsin = np.asarray(inputs["freqs_sin"], f32)

    norm_cols = np.stack([
        _cw(np.asarray(inputs["attn_norm_w"], f32)),
        _cw(np.asarray(inputs["rnn_cnorm_w"], f32)),
        _cw(np.asarray(inputs["ffn_norm_w"], f32)),
        _cw(np.asarray(inputs["r_norm_w"], f32)),
        _cw(np.asarray(inputs["lg_b1"], f32)),
        _cw(np.asarray(inputs["lg_ln_w"], f32)),
        _cw(np.asarray(inputs["lg_ln_b"], f32)),
        _cw(np.asarray(inputs["lg_b2"], f32)),
    ], axis=1)  # (128, 8, KT)

    maskT = np.zeros((128, 4, TB), f32)
    ar = np.arange(TB)
    for r in range(4):
        for k in range(128):
            maskT[k, r, :] = (128 * r + k <= ar)

    shared = {
        "cos4": np.ascontiguousarray(np.tile(fcos.T, (4, 1))).astype(BF),
        "sin4": np.ascontiguousarray(np.tile(fsin.T, (4, 1))).astype(BF),
        "maskT": maskT.astype(BF),
        "norm_cols": np.ascontiguousarray(norm_cols),
        "mgb": np.asarray(inputs["mem_gate_b"], f32).reshape(1, 1),
        "wo_t": _tw(np.asarray(inputs["wo"], f32)),
        "wog_t": _tw(np.asarray(inputs["rnn_out_w"], f32)),
        "mcomp_t": _tw(np.asarray(inputs["mem_comp_w"], f32)),
        "mq_t": _tw(np.asarray(inputs["mem_q_w"], f32)),
        "mk_t": _tw(np.asarray(inputs["mem_k_w"], f32)),
        "mv_t": _tw(np.asarray(inputs["mem_v_w"], f32)),
        "mg_t": _tw(np.asarray(inputs["mem_gate_w"], f32)),
        "lgr_t": _tw(np.asarray(inputs["lg_rnn_w"], f32)),
        "lga_t": _tw(np.asarray(inputs["lg_attn_w"], f32)),
        "w1a_t": _tw(np.asarray(inputs["lg_w1"], f32)[:, :D]),
        "w1b_t": _tw(np.asarray(inputs["lg_w1"], f32)[:, D:]),
        "w2_t": _tw(np.asarray(inputs["lg_w2"], f32)),
        "rw1_t": _tw_chunks(np.asarray(inputs["r_w1"], f32)),
        "rw3_t": _tw_chunks(np.asarray(inputs["r_w3"], f32)),
        "rw2_t": _tw_chunks(np.asarray(inputs["r_w2"], f32)),
    }

    wq = np.asarray(inputs["wq"], f32)
    wk = np.asarray(inputs["wk"], f32)
    wv = np.asarray(inputs["wv"], f32)
    wgate = np.asarray(inputs["rnn_gate_w"], f32)
    wu = np.asarray(inputs["rnn_in_w"], f32)[:D, :]
    gb = np.asarray(inputs["rnn_gate_b"], f32)

    in_maps = []
    for c in range(NC):
        perm = _head_perm(2 * c) + _head_perm(2 * c + 1)
        beta = c // 4
        bm = np.full((128, 2 * M), -480.0, f32)
        bm[:, beta * M:(beta + 1) * M] = 0.0
        xb = x[c * TB:(c + 1) * TB, :]
        m = {
            "xt": np.ascontiguousarray(
                xb.T.reshape(KT, 128, TB).transpose(1, 0, 2)).astype(f32),
            "wq_t": _tw(wq[perm, :]),
            "wk_t": _tw(wk[perm, :]),
            "wv_t": _tw(wv[2 * c * HD:(2 * c + 2) * HD, :]),
            "wg_t": _tw(wgate[128 * c:128 * (c + 1), :]),
            "wu_t": _tw(wu[128 * c:128 * (c + 1), :]),
            "gate_b": np.ascontiguousarray(
                gb[128 * c:128 * (c + 1)].reshape(128, 1)).astype(f32),
            "bmask": bm,
        }
        m.update(shared)
        in_maps.append(m)
    return in_maps


def _get_program():
    if "nc" not in _PROG_CACHE:
        _PROG_CACHE["nc"] = _build_program()
    return _PROG_CACHE["nc"]


def run_kernel_internal(inputs, **run_kwargs):
    nc = _get_program()
    in_maps = _prep_in_maps(inputs)
    res = run_bass_kernel_spmd(nc, in_maps, list(range(NC)), **run_kwargs)
    out = np.empty((NT, D), np.float32)
    for c in range(NC):
        blk = np.asarray(res.results[c]["out"], np.float32)   # (128, KT, TB)
        out[c * TB:(c + 1) * TB, :] = blk.transpose(1, 0, 2).reshape(D, TB).T
    return out.reshape(B, T, D), res


def kernel(**inputs):
    out, _ = run_kernel_internal(inputs)
    return out


# revision 13
# speedup vs baseline: 1.3568x; 1.1462x over previous
"""Trainium2 Bass kernel for nn_CoeusBlockOptimized — 8-core SPMD.

Sharding: the parallel phase is feature/head-sharded (core c owns attention
heads (2c, 2c+1) and recurrence D-shard [128c, 128c+128) for ALL tokens); the
tail is token-sharded (core c owns flattened token block [512c, 512c+512)).
Cross-core traffic: one bf16 AllGather of h^T (1 MB/rank) and one bf16
AllToAll carrying attention output + recurrence state (2 MB/rank).

All activations are kept transposed (feature rows on partitions, tokens on
the free axis); per-token reductions (rms/layernorm/softmax denominators) use
ones-vector matmuls on the PE plus a K=1 broadcast matmul back to 128
partitions.  Matmuls run in bf16 (fp32 PSUM accumulation); the sequential
recurrence uses the VectorE tensor_tensor_scan instruction in fp32.

Schedule notes: the AllGather is triggered as early as possible (only the
xt load + rms precede it); all weight loads are deferred behind it.  The
recurrence runs before attention (dense matmuls, warms the PE).  Reasoning
weights are streamed in 4 chunks with a 3-deep rotation so the PE never
waits on a whole-tensor DMA.  Row reciprocals use the fast custom-DVE
approximation (the exact InstReciprocal is ~6.5ns/elem on one partition).
"""
import sys
import os

for _p in ("/opt/trn_rl_repo", "/root/.axon_site/_ro/trn_rl_repo"):
    if os.path.isdir(_p) and _p not in sys.path:
        sys.path.insert(0, _p)

os.environ.setdefault("NEURON_RT_DBG_RDH_CC", "0")

import numpy as np
import ml_dtypes

import concourse.bass as bass
import concourse.tile as tile
from concourse import mybir, bacc
from concourse.bass_utils import run_bass_kernel_spmd
from concourse.masks import make_identity

BF = ml_dtypes.bfloat16
NPF8 = ml_dtypes.float8_e4m3
F32 = mybir.dt.float32
BF16 = mybir.dt.bfloat16
F8 = mybir.dt.float8e4
DR = mybir.MatmulPerfMode.DoubleRow
AF = mybir.ActivationFunctionType
OP = mybir.AluOpType

NC = 8
B, T, D = 2, 2048, 1024
H, HD, HID, M = 16, 64, 3072, 256
FREQ = 8
TB = 512            # tokens per core block
KT = D // 128       # 8 d-tiles
NH = HID // 128     # 24
NT = B * T          # 4096 tokens
NCH = 4             # weight streaming chunks in the reasoning block
M1 = NH // NCH      # m-tiles per w1/w3 chunk
M2 = KT // NCH      # m-tiles per w2 chunk

_PROG_CACHE = {}


def _tw(w, dt=BF):
    """(Mout, Kin) weight -> (128, Kin/128, Mout) lhsT-tile layout."""
    k, m = w.shape[1], w.shape[0]
    assert k % 128 == 0
    return np.ascontiguousarray(
        w.T.reshape(k // 128, 128, m).transpose(1, 0, 2)).astype(dt)


def _tw_chunks(w, nch=NCH, dt=BF):
    """_tw layout split into nch contiguous column chunks:
    (128, Kin/128, Mout) -> (128, nch, Kin/128, Mout/nch)."""
    t = _tw(w, dt)
    mc = t.shape[2] // nch
    return np.ascontiguousarray(
        t.reshape(128, t.shape[1], nch, mc).transpose(0, 2, 1, 3))


def _cw(v):
    """(1024,) vector -> (128, 8) fp32 per-partition column layout."""
    return np.ascontiguousarray(v.reshape(-1, 128).T).astype(np.float32)


def _head_perm(h):
    base = h * HD
    return [base + i for i in range(0, HD, 2)] + [base + i for i in range(1, HD, 2)]


def _build_program():
    nc = bacc.Bacc("TRN2", target_bir_lowering=False, debug=False, num_devices=NC)

    def din(name, shape, dt):
        return nc.dram_tensor(name, list(shape), dt, kind="ExternalInput")

    # per-core inputs
    xt = din("xt", (128, KT, TB), F32)
    wq_t = din("wq_t", (128, KT, 128), BF16)
    wk_t = din("wk_t", (128, KT, 128), BF16)
    wv_t = din("wv_t", (128, KT, 128), BF16)
    wg_t = din("wg_t", (128, KT, 128), BF16)
    wu_t = din("wu_t", (128, KT, 128), BF16)
    gate_b = din("gate_b", (128, 1), F32)
    bmask = din("bmask", (128, 2 * M), F32)
    # shared inputs
    cos4 = din("cos4", (128, T), BF16)
    sin4 = din("sin4", (128, T), BF16)
    maskT_in = din("maskT", (128, 4, TB), BF16)
    norm_cols = din("norm_cols", (128, 8, KT), F32)   # packed norm/bias columns
    mgb = din("mgb", (1, 1), F32)
    wo_t = din("wo_t", (128, KT, D), BF16)
    wog_t = din("wog_t", (128, KT, D), BF16)
    mcomp_t = din("mcomp_t", (128, KT, M), BF16)
    mq_t = din("mq_t", (128, KT, M), BF16)
    mk_t = din("mk_t", (128, 2, M), BF16)
    mv_t = din("mv_t", (128, 2, D), BF16)
    mg_t = din("mg_t", (128, KT, 1), BF16)
    lgr_t = din("lgr_t", (128, KT, D), F8)
    lga_t = din("lga_t", (128, KT, D), F8)
    w1a_t = din("w1a_t", (128, KT, D), F8)
    w1b_t = din("w1b_t", (128, KT, D), F8)
    w2_t = din("w2_t", (128, KT, D), F8)
    rw1_t = din("rw1_t", (128, NCH, KT, HID // NCH), F8)
    rw3_t = din("rw3_t", (128, NCH, KT, HID // NCH), F8)
    rw2_t = din("rw2_t", (128, NCH, NH, D // NCH), BF16)

    out_ext = nc.dram_tensor("out", [128, KT, TB], F32, kind="ExternalOutput")
    DEBUG = bool(int(os.environ.get("KERNEL_DEBUG_DUMPS", "0")))
    if DEBUG:
        dbg = {nm: nc.dram_tensor(nm, [128, KT, TB], F32, kind="ExternalOutput")
               for nm in ("d_ht0", "d_hstT", "d_atnT", "d_ol", "d_og", "d_zt",
                          "d_x2a", "d_x2b", "d_nrm0")}

    # norm_cols packing order
    NCOL = {"anorm": 0, "cnorm": 1, "ffn": 2, "rn": 3, "b1": 4, "lnw": 5,
            "lnb": 6, "b2": 7}

    with tile.TileContext(nc) as tc:
        with tc.tile_pool(name="dram", bufs=1, space="DRAM") as dram, \
             tc.tile_pool(name="const", bufs=1) as const, \
             tc.tile_pool(name="resid", bufs=1) as resid, \
             tc.tile_pool(name="tmp", bufs=3) as tmp, \
             tc.tile_pool(name="rows", bufs=4) as rows, \
             tc.tile_pool(name="ps_mm", bufs=4, space="PSUM") as ps_mm, \
             tc.tile_pool(name="ps_bc", bufs=2, space="PSUM") as ps_bc, \
             tc.tile_pool(name="ps_row", bufs=2, space="PSUM") as ps_row:

            # ---- DRAM comm buffers ----
            ag_in = dram.tile([KT, 128, TB], BF16)
            ag_out = dram.tile([NC, KT, 128, TB], BF16, addr_space="Shared")
            a2a_in = dram.tile([NC, 2, 128, TB], BF16)
            a2a_out = dram.tile([NC, 2, 128, TB], BF16)

            # ---- tiny constants needed by phase 0 ----
            ones_col = const.tile([128, 1], BF16)
            ones_row = const.tile([1, 128], BF16)
            ident = const.tile([128, 128], BF16)
            nc.any.memset(ones_col[:], 1.0)
            nc.any.memset(ones_row[:], 1.0)
            make_identity(nc, ident[:])
            ncols = const.tile([128, 8, KT], F32)
            eps6_sb = const.tile([1, 1], F32)
            eps5_sb = const.tile([1, 1], F32)
            nc.any.memset(eps6_sb[:], 1e-6)
            nc.any.memset(eps5_sb[:], 1e-5)
            nc.sync.dma_start(ncols[:], norm_cols[:])

            def ncol(nm, k):
                return ncols[:, NCOL[nm], k:k + 1]

            # ---- long-lived activations ----
            x2 = resid.tile([128, KT, TB], F32)
            atnT = resid.tile([128, KT, TB], BF16)
            hstT = resid.tile([128, KT, TB], BF16)

            # ---------- shared helpers ----------
            def rms_rsqrt_row(src_getter, eps_ap, nk=KT, width=TB):
                ss = ps_row.tile([1, width], F32, tag="psrow", name="ss")
                for k in range(nk):
                    sq = tmp.tile([128, width], BF16, tag="tb16", name="sq")
                    nc.vector.tensor_tensor(sq[:], src_getter(k), src_getter(k),
                                            OP.mult)
                    nc.tensor.matmul(ss[:], ones_col[:], sq[:],
                                     start=(k == 0), stop=(k == nk - 1))
                ms = rows.tile([1, width], F32, tag="row32", name="ms")
                nc.scalar.activation(ms[:], ss[:], AF.Identity,
                                     bias=eps_ap[0:1, 0:1], scale=1.0 / (nk * 128))
                rc = rows.tile([1, width], F32, tag="row32", name="rc")
                nc.vector.reciprocal_approx_fast(rc[:], ms[:])
                rs_row = rows.tile([1, width], BF16, tag="row16", name="rsr")
                nc.scalar.activation(rs_row[:], rc[:], AF.Sqrt)
                return rs_row

            def bcast_row(row_bf, width=TB, np_=128):
                bc = ps_bc.tile([np_, width], F32, tag="bc", name="bc")
                nc.tensor.matmul(bc[:], ones_row[0:1, 0:np_], row_bf[:])
                return bc

            with tc.tile_pool(name="ep0", bufs=1) as ep0:
                qtm = ep0.tile([128, 2, TB], BF16)
                grow = ep0.tile([1, TB], BF16)
                gbc = ep0.tile([128, TB], BF16)

                # ============== phase 0: local h^T + AllGather ==============
                with tc.tile_pool(name="htloc", bufs=1) as htloc_pool:
                    ht_loc = htloc_pool.tile([128, KT, TB], BF16)
                    with tc.tile_pool(name="ph0", bufs=1) as ph0:
                        xt_sb = ph0.tile([128, KT, TB], F32)
                        nc.sync.dma_start(xt_sb[:], xt[:])
                        rsq = rms_rsqrt_row(lambda k: xt_sb[:, k, :], eps6_sb)
                        bc = bcast_row(rsq)
                        for k in range(KT):
                            nc.vector.scalar_tensor_tensor(
                                ht_loc[:, k, :], xt_sb[:, k, :], ncol("anorm", k),
                                bc[:], OP.mult, OP.mult)
                            nc.sync.dma_start(ag_in[k], ht_loc[:, k, :])
                    nc.gpsimd.collective_compute(
                        "AllGather", OP.bypass,
                        replica_groups=[list(range(NC))],
                        ins=[ag_in.opt()], outs=[ag_out.opt()])

                    # ---- deferred constant loads (overlap the AllGather) ----
                    cos_sb = const.tile([128, T], BF16)
                    sin_sb = const.tile([128, T], BF16)
                    gate_b_sb = const.tile([128, 1], F32)
                    mgb_sb = const.tile([1, 1], F32)
                    mask_sb = const.tile([128, 4, TB], BF16)
                    bmask_sb = const.tile([128, 2 * M], F32)
                    nc.sync.dma_start(mgb_sb[:], mgb[:])
                    wq_sb = const.tile([128, KT, 128], BF16)
                    wk_sb = const.tile([128, KT, 128], BF16)
                    wv_sb = const.tile([128, KT, 128], BF16)
                    wg_sb = const.tile([128, KT, 128], BF16)
                    wu_sb = const.tile([128, KT, 128], BF16)
                    mcomp_sb = const.tile([128, KT, M], BF16)
                    mq_sb = const.tile([128, KT, M], BF16)
                    mk_sb = const.tile([128, 2, M], BF16)
                    mv_sb = const.tile([128, 2, D], BF16)
                    mg_sb = const.tile([128, KT, 1], BF16)
                    for sb, t_in in ((mq_sb, mq_t), (mg_sb, mg_t), (wg_sb, wg_t),
                                     (wu_sb, wu_t), (wq_sb, wq_t), (wk_sb, wk_t),
                                     (wv_sb, wv_t), (mcomp_sb, mcomp_t),
                                     (mk_sb, mk_t), (mv_sb, mv_t)):
                        nc.sync.dma_start(sb[:], t_in[:])
                    nc.sync.dma_start(gate_b_sb[:], gate_b[:])
                    nc.sync.dma_start(cos_sb[:], cos4[:])
                    nc.sync.dma_start(sin_sb[:], sin4[:])
                    nc.sync.dma_start(mask_sb[:], maskT_in[:])
                    nc.sync.dma_start(bmask_sb[:], bmask[:])

                    # episodic local-only pieces run during the AllGather
                    for mi in range(2):
                        ps = ps_mm.tile([128, TB], F32, tag="mm", name="q_ps")
                        for k in range(KT):
                            nc.tensor.matmul(
                                ps[:],
                                mq_sb[:, k, mi * 128:(mi + 1) * 128],
                                ht_loc[:, k, :], start=(k == 0),
                                stop=(k == KT - 1))
                        nc.scalar.copy(qtm[:, mi, :], ps[:])
                    psg2 = ps_row.tile([1, TB], F32, tag="psrow", name="g_psr")
                    for k in range(KT):
                        nc.tensor.matmul(psg2[:], mg_sb[:, k, :],
                                         ht_loc[:, k, :],
                                         start=(k == 0), stop=(k == KT - 1))
                    nc.scalar.activation(grow[:], psg2[:], AF.Sigmoid,
                                         bias=mgb_sb[0:1, 0:1])
                    gb = bcast_row(grow)
                    nc.scalar.copy(gbc[:], gb[:])

                with tc.tile_pool(name="ht", bufs=1) as ht_pool:
                    ht = ht_pool.tile([128, KT, NC, TB], BF16)
                    for k in range(KT):
                        nc.sync.dma_start(
                            ht[:, k, :, :],
                            ag_out[:, k, :, :].rearrange("blk p t -> p blk t"))
                    if DEBUG:
                        nc.gpsimd.dma_start(dbg["d_ht0"][:], ht[:, :, 0, :])

                    # ========= phase 1: recurrence (dense, runs first) =========
                    with tc.tile_pool(name="scan", bufs=1) as scan_pool:
                        gate_sb = scan_pool.tile([128, NC, TB], F32)
                        u_sb = scan_pool.tile([128, NC, TB], F32)
                        hst_sb = scan_pool.tile([128, NC * TB], BF16)
                        for blk in range(NC):
                            psg = ps_mm.tile([128, TB], F32, tag="mm",
                                             name="g_ps")
                            for k in range(KT):
                                nc.tensor.matmul(psg[:], wg_sb[:, k, :],
                                                 ht[:, k, blk, :],
                                                 start=(k == 0),
                                                 stop=(k == KT - 1))
                            nc.scalar.activation(gate_sb[:, blk, :], psg[:],
                                                 AF.Sigmoid,
                                                 bias=gate_b_sb[:])
                            psu = ps_mm.tile([128, TB], F32, tag="mm",
                                             name="u_ps")
                            for k in range(KT):
                                nc.tensor.matmul(psu[:], wu_sb[:, k, :],
                                                 ht[:, k, blk, :],
                                                 start=(k == 0),
                                                 stop=(k == KT - 1))
                            nc.scalar.activation(u_sb[:, blk, :], psu[:],
                                                 AF.Silu)
                        g2d = gate_sb[:].rearrange("p a b -> p (a b)")
                        u2d = u_sb[:].rearrange("p a b -> p (a b)")
                        for b in range(B):
                            cols = slice(b * T, (b + 1) * T)
                            nc.vector.tensor_tensor_scan(
                                hst_sb[:, cols], g2d[:, cols], u2d[:, cols],
                                0.0, OP.mult, OP.add)
                        for j in range(NC):
                            nc.sync.dma_start(
                                a2a_in[j, 1], hst_sb[:, j * TB:(j + 1) * TB])

                    # ==================== phase 2: attention ====================
                    with tc.tile_pool(name="attn", bufs=1) as attn:
                        for b in range(B):
                            qT = attn.tile([128, T], BF16, tag="qT",
                                           name=f"qT{b}")
                            kTt = attn.tile([128, T], BF16, tag="kT",
                                            name=f"kT{b}")
                            vn = attn.tile([128, 16, 130], BF16, tag="vn",
                                           name=f"vn{b}")
                            nc.any.memset(vn[:, :, 64:65], 1.0)
                            nc.any.memset(vn[:, :, 129:130], 1.0)

                            for dst, wsb in ((qT, wq_sb), (kTt, wk_sb)):
                                for nb in range(4):
                                    cols = slice(nb * TB, (nb + 1) * TB)
                                    ps = ps_mm.tile([128, TB], F32, tag="mm",
                                                    name="qk_ps")
                                    for k in range(KT):
                                        nc.tensor.matmul(
                                            ps[:], wsb[:, k, :],
                                            ht[:, k, 4 * b + nb, :],
                                            start=(k == 0),
                                            stop=(k == KT - 1))
                                    m1 = tmp.tile([128, TB], F32, tag="tf32",
                                                  name="rot1")
                                    m2 = ps_bc.tile([128, TB], F32, tag="bc",
                                                    name="rot2")
                                    nc.vector.tensor_tensor(
                                        m1[:], ps[:], cos_sb[:, cols], OP.mult)
                                    nc.vector.tensor_tensor(
                                        m2[:], ps[:], sin_sb[:, cols], OP.mult)
                                    for h2 in range(2):
                                        be = 64 * h2
                                        nc.vector.tensor_tensor(
                                            dst[be:be + 32, cols],
                                            m1[be:be + 32, :],
                                            m2[be + 32:be + 64, :],
                                            OP.subtract)
                                        nc.vector.tensor_tensor(
                                            dst[be + 32:be + 64, cols],
                                            m1[be + 32:be + 64, :],
                                            m2[be:be + 32, :], OP.add)

                            for tt in range(16):
                                nb, sub = tt // 4, tt % 4
                                ps = ps_mm.tile([128, 128], F32, tag="mm",
                                                name="v_ps")
                                for k in range(KT):
                                    nc.tensor.matmul(
                                        ps[:],
                                        ht[:, k, 4 * b + nb,
                                           sub * 128:(sub + 1) * 128],
                                        wv_sb[:, k, :],
                                        start=(k == 0), stop=(k == KT - 1))
                                nc.scalar.copy(vn[:, tt, 0:64], ps[:, 0:64])
                                nc.scalar.copy(vn[:, tt, 65:129],
                                               ps[:, 64:128])

                            for h2 in range(2):
                                be = 64 * h2
                                for qb in range(4):
                                    qcols = slice(qb * TB, (qb + 1) * TB)
                                    nlive = 4 * qb + 4
                                    pt = attn.tile([128, 16, TB], BF16,
                                                   tag="pt", name="pt",
                                                   bufs=2)
                                    for ki in range(nlive):
                                        sps = ps_mm.tile([128, TB], F32,
                                                         tag="mm", name="s_ps")
                                        nc.tensor.matmul(
                                            sps[:],
                                            kTt[be:be + 64,
                                                ki * 128:(ki + 1) * 128],
                                            qT[be:be + 64, qcols],
                                            start=True, stop=True)
                                        nc.scalar.activation(
                                            pt[:, ki, :], sps[:], AF.Exp,
                                            scale=0.125)
                                        if ki >= 4 * qb:
                                            nc.vector.tensor_tensor(
                                                pt[:, ki, :], pt[:, ki, :],
                                                mask_sb[:, ki - 4 * qb, :],
                                                OP.mult)
                                    av = ps_mm.tile([65, TB], F32, tag="mm",
                                                    name="av_ps")
                                    for ki in range(nlive):
                                        nc.tensor.matmul(
                                            av[:],
                                            vn[:, ki, 65 * h2:65 * h2 + 65],
                                            pt[:, ki, :], start=(ki == 0),
                                            stop=(ki == nlive - 1))
                                    dnr = rows.tile([1, TB], F32, tag="row32",
                                                    name="dnr")
                                    nc.scalar.copy(dnr[:], av[64:65, :])
                                    rd = rows.tile([1, TB], F32, tag="row32",
                                                   name="rd")
                                    nc.vector.reciprocal_approx_fast(
                                        rd[:], dnr[:])
                                    rdb = rows.tile([1, TB], BF16,
                                                    tag="row16", name="rdb")
                                    nc.scalar.copy(rdb[:], rd[:])
                                    bcd = bcast_row(rdb, np_=64)
                                    bsb = tmp.tile([64, TB], BF16, tag="t64",
                                                   name="bsb", bufs=4)
                                    nc.scalar.copy(bsb[:], bcd[:])
                                    an_t = tmp.tile([64, TB], BF16,
                                                    tag="t64", name="an_t",
                                                    bufs=4)
                                    nc.vector.tensor_tensor(
                                        an_t[:], av[0:64, :], bsb[:], OP.mult)
                                    nc.sync.dma_start(
                                        a2a_in[4 * b + qb, 0,
                                               be:be + 64, :], an_t[:])

                    nc.gpsimd.collective_compute(
                        "AllToAll", OP.bypass,
                        replica_groups=[list(range(NC))],
                        ins=[a2a_in.opt()], outs=[a2a_out.opt()])
                    # tail inputs gathered as soon as the A2A lands
                    nc.sync.dma_start(atnT[:],
                                      a2a_out[:, 0].rearrange("i p t -> p i t"))
                    nc.sync.dma_start(hstT[:],
                                      a2a_out[:, 1].rearrange("i p t -> p i t"))
                    if DEBUG:
                        nc.gpsimd.dma_start(dbg["d_atnT"][:], atnT[:])
                        nc.gpsimd.dma_start(dbg["d_hstT"][:], hstT[:])

                    # ========= phase 3: episodic (overlaps the A2A) =========
                    with tc.tile_pool(name="ep", bufs=1) as ep:
                        memk = ep.tile([128, 2, 2 * M], BF16)
                        ktm = ep.tile([128, 2, 2 * M], BF16)
                        a_sb = ep.tile([128, 4, 2 * M], BF16)
                        at_sb = ep.tile([128, 4, TB], BF16)
                        sn_sb = ep.tile([128, 4, D], BF16)
                        mo_sb = ep.tile([128, KT, TB], BF16)
                        moc = ep.tile([128, 2, TB], BF16)

                        for mi in range(2):
                            ps = ps_mm.tile([128, 2 * M], F32, tag="mm",
                                            name="mk_ps")
                            for k in range(KT):
                                nc.tensor.matmul(
                                    ps[:],
                                    mcomp_sb[:, k, mi * 128:(mi + 1) * 128],
                                    ht[:, k, :, 0:TB:FREQ],
                                    start=(k == 0), stop=(k == KT - 1))
                            nc.scalar.copy(memk[:, mi, :], ps[:])
                        for mo in range(2):
                            ps = ps_mm.tile([128, 2 * M], F32, tag="mm",
                                            name="kt_ps")
                            for mi in range(2):
                                nc.tensor.matmul(
                                    ps[:],
                                    mk_sb[:, mi, mo * 128:(mo + 1) * 128],
                                    memk[:, mi, :], start=(mi == 0),
                                    stop=(mi == 1))
                            nc.scalar.copy(ktm[:, mo, :], ps[:])
                        for tt in range(4):
                            ps = ps_mm.tile([128, 2 * M], F32, tag="mm",
                                            name="sc_ps")
                            for mi in range(2):
                                nc.tensor.matmul(
                                    ps[:],
                                    qtm[:, mi, tt * 128:(tt + 1) * 128],
                                    ktm[:, mi, :], start=(mi == 0),
                                    stop=(mi == 1))
                            sm = tmp.tile([128, 2 * M], F32, tag="tf32",
                                          name="sm")
                            nc.vector.tensor_tensor(sm[:], ps[:], bmask_sb[:],
                                                    OP.add)
                            den = rows.tile([128, 1], F32, tag="den",
                                            name="den")
                            nc.scalar.activation(a_sb[:, tt, :], sm[:],
                                                 AF.Exp, scale=1.0 / 16.0,
                                                 accum_out=den[:])
                            rden = rows.tile([128, 1], F32, tag="den",
                                             name="rden")
                            nc.vector.reciprocal(rden[:], den[:])
                            nc.vector.tensor_scalar_mul(
                                a_sb[:, tt, :], a_sb[:, tt, :], rden[:])
                        for st in range(4):
                            for k in range(KT):
                                pst = ps_mm.tile([128, 128], BF16, tag="mm",
                                                 name="tr_ps")
                                nc.tensor.transpose(
                                    pst[:],
                                    ht[:, k, 2 * st:2 * st + 2, 0:TB:FREQ],
                                    ident[:])
                                nc.scalar.copy(
                                    sn_sb[:, st, k * 128:(k + 1) * 128],
                                    pst[:])
                            for tt in range(4):
                                pst = ps_mm.tile([128, 128], BF16, tag="mm",
                                                 name="tr2_ps")
                                nc.tensor.transpose(
                                    pst[:],
                                    a_sb[:, tt, st * 128:(st + 1) * 128],
                                    ident[:])
                                nc.scalar.copy(
                                    at_sb[:, st, tt * 128:(tt + 1) * 128],
                                    pst[:])
                        for dm in range(KT):
                            ps = ps_mm.tile([128, TB], F32, tag="mm",
                                            name="mo_ps")
                            for st in range(4):
                                nc.tensor.matmul(
                                    ps[:],
                                    sn_sb[:, st, dm * 128:(dm + 1) * 128],
                                    at_sb[:, st, :], start=(st == 0),
                                    stop=(st == 3))
                            nc.scalar.copy(mo_sb[:, dm, :], ps[:])
                        for mi in range(2):
                            ps = ps_mm.tile([128, TB], F32, tag="mm",
                                            name="moc_ps")
                            for k in range(KT):
                                nc.tensor.matmul(
                                    ps[:],
                                    mcomp_sb[:, k, mi * 128:(mi + 1) * 128],
                                    mo_sb[:, k, :], start=(k == 0),
                                    stop=(k == KT - 1))
                            nc.scalar.copy(moc[:, mi, :], ps[:])
                        # x2 = x + out_episodic (residual base, pre-tail)
                        for dm in range(KT):
                            ps = ps_mm.tile([128, TB], F32, tag="mm",
                                            name="mv_ps")
                            for mi in range(2):
                                nc.tensor.matmul(
                                    ps[:],
                                    mv_sb[:, mi, dm * 128:(dm + 1) * 128],
                                    moc[:, mi, :], start=(mi == 0),
                                    stop=(mi == 1))
                            e1 = tmp.tile([128, TB], BF16, tag="tb16",
                                          name="e1")
                            nc.vector.tensor_tensor(e1[:], ps[:], gbc[:],
                                                    OP.mult)
                            xrk = tmp.tile([128, TB], F32, tag="tf32",
                                           name="xrk")
                            nc.sync.dma_start(xrk[:], xt[:, dm, :])
                            nc.vector.tensor_tensor(x2[:, dm, :], xrk[:],
                                                    e1[:], OP.add)
                        if DEBUG:
                            nc.gpsimd.dma_start(dbg["d_x2a"][:], x2[:])

            # ================= phase 4: tail (token-parallel) =================
            with tc.tile_pool(name="tlg", bufs=1) as tlg, \
                 tc.tile_pool(name="wt", bufs=2) as wt:

                def mm_chain(w_dram, rhs_fn, evict, nk=KT, nm=KT, dr=False):
                    wdt = F8 if dr else BF16
                    wsb = wt.tile([128, nk, D], wdt, tag="w", name="wstream")
                    nc.sync.dma_start(wsb[:], w_dram[:])
                    for m in range(nm):
                        ps = ps_mm.tile([128, TB], F32, tag="mm", name="c_ps")
                        if dr:
                            for j in range(nk // 2):
                                nc.tensor.matmul(
                                    ps[:],
                                    wsb[:, 2 * j:2 * j + 2,
                                        m * 128:(m + 1) * 128],
                                    rhs_fn(j), start=(j == 0),
                                    stop=(j == nk // 2 - 1), perf_mode=DR)
                        else:
                            for k in range(nk):
                                nc.tensor.matmul(
                                    ps[:], wsb[:, k, m * 128:(m + 1) * 128],
                                    rhs_fn(k), start=(k == 0),
                                    stop=(k == nk - 1))
                        evict(m, ps)

                ol = tlg.tile([128, KT, TB], BF16)
                ol8 = tlg.tile([128, KT, TB], F8)

                def ev_ol(m, ps):
                    nc.scalar.copy(ol[:, m, :], ps[:])
                    nc.vector.tensor_copy(ol8[:, m, :], ps[:])
                mm_chain(wo_t, lambda k: atnT[:, k, :], ev_ol)
                if DEBUG:
                    nc.gpsimd.dma_start(dbg["d_ol"][:], ol[:])
                hatt = tlg.tile([128, KT, TB], F8, tag="hio",
                                name="hatt", bufs=2)
                mm_chain(lga_t, lambda j: ol8[:, 2 * j:2 * j + 2, :],
                         lambda m, ps: nc.scalar.copy(hatt[:, m, :], ps[:]),
                         dr=True)

                nT = tlg.tile([128, KT, TB], BF16, tag="nT", name="nT")
                rsq2 = rms_rsqrt_row(lambda k: hstT[:, k, :], eps6_sb)
                bc2 = bcast_row(rsq2)
                for k in range(KT):
                    nc.vector.scalar_tensor_tensor(
                        nT[:, k, :], hstT[:, k, :], ncol("cnorm", k), bc2[:],
                        OP.mult, OP.mult)
                og = tlg.tile([128, KT, TB], BF16)
                og8 = tlg.tile([128, KT, TB], F8)

                def ev_og(m, ps):
                    nc.scalar.copy(og[:, m, :], ps[:])
                    nc.vector.tensor_copy(og8[:, m, :], ps[:])
                mm_chain(wog_t, lambda k: nT[:, k, :], ev_og)
                if DEBUG:
                    nc.gpsimd.dma_start(dbg["d_og"][:], og[:])
                hrnn = tlg.tile([128, KT, TB], F8, tag="hio",
                                name="hrnn", bufs=2)
                mm_chain(lgr_t, lambda j: og8[:, 2 * j:2 * j + 2, :],
                         lambda m, ps: nc.scalar.copy(hrnn[:, m, :], ps[:]),
                         dr=True)

                t1b = tlg.tile([128, KT, TB], BF16, tag="t1b", name="t1b")
                w1a_sb = wt.tile([128, KT, D], F8, tag="w", name="w1a_sb")
                w1b_sb = wt.tile([128, KT, D], F8, tag="w", name="w1b_sb")
                nc.sync.dma_start(w1b_sb[:], w1b_t[:])
                nc.sync.dma_start(w1a_sb[:], w1a_t[:])
                for m in range(KT):
                    ps = ps_mm.tile([128, TB], F32, tag="mm", name="t1_ps")
                    for j in range(KT // 2):
                        nc.tensor.matmul(
                            ps[:],
                            w1b_sb[:, 2 * j:2 * j + 2, m * 128:(m + 1) * 128],
                            hatt[:, 2 * j:2 * j + 2, :], start=(j == 0),
                            stop=False, perf_mode=DR)
                    for j in range(KT // 2):
                        nc.tensor.matmul(
                            ps[:],
                            w1a_sb[:, 2 * j:2 * j + 2, m * 128:(m + 1) * 128],
                            hrnn[:, 2 * j:2 * j + 2, :], start=False,
                            stop=(j == KT // 2 - 1), perf_mode=DR)
                    nc.scalar.activation(t1b[:, m, :], ps[:], AF.Identity,
                                         bias=ncol("b1", m))

                # layernorm stats (mean via ones-matmul; fast reciprocal)
                ssum = ps_row.tile([1, TB], F32, tag="psrow", name="ssum")
                for k in range(KT):
                    nc.tensor.matmul(ssum[:], ones_col[:], t1b[:, k, :],
                                     start=(k == 0), stop=(k == KT - 1))
                ssq = ps_row.tile([1, TB], F32, tag="psrow", name="ssq")
                for k in range(KT):
                    sq = tmp.tile([128, TB], BF16, tag="tb16", name="sq2")
                    nc.vector.tensor_tensor(sq[:], t1b[:, k, :], t1b[:, k, :],
                                            OP.mult)
                    nc.tensor.matmul(ssq[:], ones_col[:], sq[:],
                                     start=(k == 0), stop=(k == KT - 1))
                mu = rows.tile([1, TB], F32, tag="row32", name="mu")
                nc.scalar.activation(mu[:], ssum[:], AF.Identity, scale=1.0 / D)
                mub = rows.tile([1, TB], BF16, tag="row16", name="mub")
                nc.scalar.copy(mub[:], mu[:])
                mu2 = rows.tile([1, TB], F32, tag="row32", name="mu2")
                nc.vector.tensor_tensor(mu2[:], mu[:], mu[:], OP.mult)
                ex2 = rows.tile([1, TB], F32, tag="row32", name="ex2")
                nc.scalar.activation(ex2[:], ssq[:], AF.Identity, scale=1.0 / D)
                varr = rows.tile([1, TB], F32, tag="row32", name="varr")
                nc.vector.tensor_tensor(varr[:], ex2[:], mu2[:], OP.subtract)
                vre = rows.tile([1, TB], F32, tag="row32", name="vre")
                nc.scalar.activation(vre[:], varr[:], AF.Identity,
                                     bias=eps5_sb[0:1, 0:1])
                rcv = rows.tile([1, TB], F32, tag="row32", name="rcv")
                nc.vector.reciprocal_approx_fast(rcv[:], vre[:])
                rsl = rows.tile([1, TB], BF16, tag="row16", name="rsl")
                nc.scalar.activation(rsl[:], rcv[:], AF.Sqrt)
                bc_mu = bcast_row(mub)
                bc_rs = bcast_row(rsl)
                zt = tlg.tile([128, KT, TB], F8, tag="zt", name="zt")
                for k in range(KT):
                    d1 = tmp.tile([128, TB], F32, tag="tf32", name="d1")
                    nc.vector.tensor_tensor(d1[:], t1b[:, k, :], bc_mu[:],
                                            OP.subtract)
                    d2 = tmp.tile([128, TB], F32, tag="tf32", name="d2")
                    nc.vector.tensor_tensor(d2[:], d1[:], bc_rs[:], OP.mult)
                    nc.scalar.activation(zt[:, k, :], d2[:], AF.Silu,
                                         bias=ncol("lnb", k),
                                         scale=ncol("lnw", k))
                if DEBUG:
                    nc.gpsimd.dma_start(dbg["d_zt"][:], zt[:])
                g2t = tlg.tile([128, KT, TB], BF16)
                mm_chain(w2_t, lambda j: zt[:, 2 * j:2 * j + 2, :],
                         lambda m, ps: nc.scalar.activation(
                             g2t[:, m, :], ps[:], AF.Sigmoid,
                             bias=ncol("b2", m)), dr=True)

                for k in range(KT):
                    mx1 = tmp.tile([128, TB], F32, tag="tf32", name="mx1")
                    nc.vector.tensor_tensor(mx1[:], ol[:, k, :], og[:, k, :],
                                            OP.subtract)
                    mx2 = tmp.tile([128, TB], F32, tag="tf32", name="mx2")
                    nc.vector.tensor_tensor(mx2[:], g2t[:, k, :], mx1[:],
                                            OP.mult)
                    mx3 = tmp.tile([128, TB], F32, tag="tf32", name="mx3")
                    nc.vector.tensor_tensor(mx3[:], og[:, k, :], mx2[:], OP.add)
                    nc.vector.tensor_tensor(x2[:, k, :], x2[:, k, :], mx3[:],
                                            OP.add)

            if DEBUG:
                nc.gpsimd.dma_start(dbg["d_x2b"][:], x2[:])
            # ================= phase 5: reasoning (SwiGLU x2) =================
            with tc.tile_pool(name="trs", bufs=1) as trs, \
                 tc.tile_pool(name="wr", bufs=1) as wr:
                rs = trs.tile([128, KT, TB], F32)
                rsq3 = rms_rsqrt_row(lambda k: x2[:, k, :], eps6_sb)
                bc3 = bcast_row(rsq3)
                for k in range(KT):
                    nc.vector.scalar_tensor_tensor(
                        rs[:, k, :], x2[:, k, :], ncol("ffn", k), bc3[:],
                        OP.mult, OP.mult)
                for it in range(2):
                    nrm = trs.tile([128, KT, TB], F8, tag="nrm",
                                   name=f"nrm{it}")
                    rsq4 = rms_rsqrt_row(lambda k: rs[:, k, :], eps6_sb)
                    bc4 = bcast_row(rsq4)
                    for k in range(KT):
                        nc.vector.scalar_tensor_tensor(
                            nrm[:, k, :], rs[:, k, :], ncol("rn", k), bc4[:],
                            OP.mult, OP.mult)
                    if DEBUG and it == 0:
                        nc.gpsimd.dma_start(dbg["d_nrm0"][:], nrm[:])
                    asb = trs.tile([128, NH, TB], BF16, tag="asb",
                                   name=f"asb{it}")
                    for c in range(NCH):
                        w1c = wr.tile([128, KT, HID // NCH], F8, tag="rwc",
                                      name="w1c", bufs=3)
                        nc.sync.dma_start(w1c[:], rw1_t[:, c])
                        for m6 in range(M1):
                            m = c * M1 + m6
                            ps = ps_mm.tile([128, TB], F32, tag="mm",
                                            name="a_ps")
                            for j in range(KT // 2):
                                nc.tensor.matmul(
                                    ps[:],
                                    w1c[:, 2 * j:2 * j + 2,
                                        m6 * 128:(m6 + 1) * 128],
                                    nrm[:, 2 * j:2 * j + 2, :],
                                    start=(j == 0), stop=(j == KT // 2 - 1),
                                    perf_mode=DR)
                            nc.scalar.activation(asb[:, m, :], ps[:], AF.Silu)
                    absb = trs.tile([128, NH, TB], BF16, tag="absb",
                                    name=f"absb{it}")
                    for c in range(NCH):
                        w3c = wr.tile([128, KT, HID // NCH], F8, tag="rwc",
                                      name="w3c", bufs=3)
                        nc.sync.dma_start(w3c[:], rw3_t[:, c])
                        for m6 in range(M1):
                            m = c * M1 + m6
                            ps = ps_mm.tile([128, TB], F32, tag="mm",
                                            name="b_ps")
                            for j in range(KT // 2):
                                nc.tensor.matmul(
                                    ps[:],
                                    w3c[:, 2 * j:2 * j + 2,
                                        m6 * 128:(m6 + 1) * 128],
                                    nrm[:, 2 * j:2 * j + 2, :],
                                    start=(j == 0), stop=(j == KT // 2 - 1),
                                    perf_mode=DR)
                            nc.vector.tensor_tensor(absb[:, m, :], ps[:],
                                                    asb[:, m, :], OP.mult)
                    for c in range(NCH):
                        w2c = wr.tile([128, NH, D // NCH], BF16, tag="rwc",
                                      name="w2c", bufs=3)
                        nc.sync.dma_start(w2c[:], rw2_t[:, c])
                        for m2 in range(M2):
                            m = c * M2 + m2
                            ps = ps_mm.tile([128, TB], F32, tag="mm",
                                            name="o_ps")
                            for k in range(NH):
                                nc.tensor.matmul(
                                    ps[:], w2c[:, k, m2 * 128:(m2 + 1) * 128],
                                    absb[:, k, :], start=(k == 0),
                                    stop=(k == NH - 1))
                            nc.vector.tensor_tensor(rs[:, m, :], ps[:],
                                                    rs[:, m, :], OP.add)
                            if it == 1:
                                # final: out = x2 + rs, folded into the evict
                                fo = tmp.tile([128, TB], F32, tag="tf32",
                                              name="fo")
                                nc.vector.tensor_tensor(fo[:], x2[:, m, :],
                                                        rs[:, m, :], OP.add)
                                nc.sync.dma_start(out_ext[:, m, :], fo[:])

    nc.compile()
    return nc


def _prep_in_maps(inputs):
    f32 = np.float32
    x = np.asarray(inputs["x"], f32).reshape(NT, D)
    fcos = np.asarray(inputs["freqs_cos"], f32)
    fsin = np.asarray(inputs["freqs_sin"], f32)

    norm_cols = np.stack([
        _cw(np.asarray(inputs["attn_norm_w"], f32)),
        _cw(np.asarray(inputs["rnn_cnorm_w"], f32)),
        _cw(np.asarray(inputs["ffn_norm_w"], f32)),
        _cw(np.asarray(inputs["r_norm_w"], f32)),
        _cw(np.asarray(inputs["lg_b1"], f32)),
        _cw(np.asarray(inputs["lg_ln_w"], f32)),
        _cw(np.asarray(inputs["lg_ln_b"], f32)),
        _cw(np.asarray(inputs["lg_b2"], f32)),
    ], axis=1)  # (128, 8, KT)

    maskT = np.zeros((128, 4, TB), f32)
    ar = np.arange(TB)
    for r in range(4):
        for k in range(128):
            maskT[k, r, :] = (128 * r + k <= ar)

    shared = {
        "cos4": np.ascontiguousarray(np.tile(fcos.T, (4, 1))).astype(BF),
        "sin4": np.ascontiguousarray(np.tile(fsin.T, (4, 1))).astype(BF),
        "maskT": maskT.astype(BF),
        "norm_cols": np.ascontiguousarray(norm_cols),
        "mgb": np.asarray(inputs["mem_gate_b"], f32).reshape(1, 1),
        "wo_t": _tw(np.asarray(inputs["wo"], f32)),
        "wog_t": _tw(np.asarray(inputs["rnn_out_w"], f32)),
        "mcomp_t": _tw(np.asarray(inputs["mem_comp_w"], f32)),
        "mq_t": _tw(np.asarray(inputs["mem_q_w"], f32)),
        "mk_t": _tw(np.asarray(inputs["mem_k_w"], f32)),
        "mv_t": _tw(np.asarray(inputs["mem_v_w"], f32)),
        "mg_t": _tw(np.asarray(inputs["mem_gate_w"], f32)),
        "lgr_t": _tw(np.asarray(inputs["lg_rnn_w"], f32), NPF8),
        "lga_t": _tw(np.asarray(inputs["lg_attn_w"], f32), NPF8),
        "w1a_t": _tw(np.asarray(inputs["lg_w1"], f32)[:, :D], NPF8),
        "w1b_t": _tw(np.asarray(inputs["lg_w1"], f32)[:, D:], NPF8),
        "w2_t": _tw(np.asarray(inputs["lg_w2"], f32), NPF8),
        "rw1_t": _tw_chunks(np.asarray(inputs["r_w1"], f32), dt=NPF8),
        "rw3_t": _tw_chunks(np.asarray(inputs["r_w3"], f32), dt=NPF8),
        "rw2_t": _tw_chunks(np.asarray(inputs["r_w2"], f32)),
    }

    wq = np.asarray(inputs["wq"], f32)
    wk = np.asarray(inputs["wk"], f32)
    wv = np.asarray(inputs["wv"], f32)
    wgate = np.asarray(inputs["rnn_gate_w"], f32)
    wu = np.asarray(inputs["rnn_in_w"], f32)[:D, :]
    gb = np.asarray(inputs["rnn_gate_b"], f32)

    in_maps = []
    for c in range(NC):
        perm = _head_perm(2 * c) + _head_perm(2 * c + 1)
        beta = c // 4
        bm = np.full((128, 2 * M), -480.0, f32)
        bm[:, beta * M:(beta + 1) * M] = 0.0
        xb = x[c * TB:(c + 1) * TB, :]
        m = {
            "xt": np.ascontiguousarray(
                xb.T.reshape(KT, 128, TB).transpose(1, 0, 2)).astype(f32),
            "wq_t": _tw(wq[perm, :]),
            "wk_t": _tw(wk[perm, :]),
            "wv_t": _tw(wv[2 * c * HD:(2 * c + 2) * HD, :]),
            "wg_t": _tw(wgate[128 * c:128 * (c + 1), :]),
            "wu_t": _tw(wu[128 * c:128 * (c + 1), :]),
            "gate_b": np.ascontiguousarray(
                gb[128 * c:128 * (c + 1)].reshape(128, 1)).astype(f32),
            "bmask": bm,
        }
        m.update(shared)
        in_maps.append(m)
    return in_maps


def _get_program():
    if "nc" not in _PROG_CACHE:
        _PROG_CACHE["nc"] = _build_program()
    return _PROG_CACHE["nc"]


def run_kernel_internal(inputs, **run_kwargs):
    nc = _get_program()
    in_maps = _prep_in_maps(inputs)
    res = run_bass_kernel_spmd(nc, in_maps, list(range(NC)), **run_kwargs)
    out = np.empty((NT, D), np.float32)
    for c in range(NC):
        blk = np.asarray(res.results[c]["out"], np.float32)   # (128, KT, TB)
        out[c * TB:(c + 1) * TB, :] = blk.transpose(1, 0, 2).reshape(D, TB).T
    return out.reshape(B, T, D), res


def kernel(**inputs):
    out, _ = run_kernel_internal(inputs)
    return out
